# revision 1
# baseline (speedup 1.0000x reference)
"""Trainium2 Bass kernel for nn_DispersionInteraction (vdW-QDO dispersion).

Strategy (8 NeuronCores, SPMD single NEFF):
  - Edges are sharded across cores by RECEIVER block (core c owns nodes
    [c*12500, (c+1)*12500)), so each core's local segment-sum covers only
    12544 bins and no cross-core reduction is needed (outputs concatenate).
  - Node phase (per core): builds the (alpha_n, C6_n) = (A[z]*h, C[z]*h^2)
    table fully on-device. The 100-entry element tables are gathered via a
    one-hot matmul on the tensor engine (z broadcast -> is_equal vs
    partition iota -> matmul against the [128,2] constant table).
  - Gather phase (raw, non-Tile): per-edge (alpha, C6) records for sender
    and receiver are fetched with the GPSIMD dma_gather ucode op at
    32-node block granularity (256B rows, int16 block ids), then the right
    8B record is selected on the vector engine with a one-hot over the
    low 5 index bits. (Generic indirect DMA is broken in this toolchain:
    "DynamicDMA is disabled" - dma_gather is the only HW-correct gather.)
  - Edge phase (Tile): per-edge energies via DVE/ACT ops; the segment-sum
    runs on the tensor engine via one-hot matmuls accumulating into a
    PSUM [128, 98] bin grid (bin = (r_local & 127, r_local >> 7)).
  - Runs as two NEFFs (node table bounced through host) since the gather
    needs the table as a plain ExternalInput.
"""

import math
import sys

import numpy as np

sys.path.insert(0, "/opt/trn_rl_repo")

import concourse.bass as bass
import concourse.tile as tile
from concourse import bacc, mybir
from concourse.bass_utils import run_bass_kernel_spmd
from contextlib import ExitStack

F32 = mybir.dt.float32
I32 = mybir.dt.int32

BOHR = 0.5291772105638411
FINE_STRUCTURE = 0.0072973525693
HARTREE = 27.211386245988
C_FACTOR = 0.5

ALPHAS = np.array([4.5, 1.38, 164.2, 38.0, 21.0, 12.0, 7.4, 5.4, 3.8, 2.67, 162.7, 71.0, 60.0, 37.0, 25.0, 19.6, 15.0, 11.1, 292.9, 160.0, 120.0, 98.0, 84.0, 78.0, 63.0, 56.0, 50.0, 48.0, 42.0, 40.0, 60.0, 41.0, 29.0, 25.0, 20.0, 16.8, 319.2, 199.0, 126.74, 119.97, 101.6, 88.42, 80.08, 65.89, 56.1, 23.68, 50.6, 39.7, 70.22, 55.95, 43.67, 37.65, 35.0, 27.3, 399.9, 275.0, 213.7, 204.7, 215.8, 208.4, 200.2, 192.1, 184.2, 158.3, 169.5, 164.64, 156.3, 150.2, 144.3, 138.9, 137.2, 99.52, 82.53, 71.04, 63.04, 55.06, 42.51, 39.68, 36.5, 33.9, 69.92, 61.8, 49.02, 45.01, 38.93, 33.54, 317.8, 246.2, 203.3, 217.0, 154.4, 127.8, 150.5, 132.2, 131.2, 143.6, 125.3, 121.5, 117.5, 113.4, 109.4, 105.4], dtype=np.float32)
C6_COEF = np.array([6.5, 1.46, 1387.0, 214.0, 99.5, 46.6, 24.2, 15.6, 9.52, 6.38, 1556.0, 627.0, 528.0, 305.0, 185.0, 134.0, 94.6, 64.3, 3897.0, 2221.0, 1383.0, 1044.0, 832.0, 602.0, 552.0, 482.0, 408.0, 373.0, 253.0, 284.0, 498.0, 354.0, 246.0, 210.0, 162.0, 129.6, 4691.0, 3170.0, 1968.58, 1677.91, 1263.61, 1028.73, 1390.87, 609.75, 469.0, 157.5, 339.0, 452.0, 707.05, 587.42, 459.32, 396.0, 385.0, 285.9, 6846.0, 5727.0, 3884.5, 3708.33, 3911.84, 3908.75, 3847.68, 3708.69, 3511.71, 2781.53, 3124.41, 2984.29, 2839.95, 2724.12, 2576.78, 2387.53, 2371.8, 1274.8, 1019.92, 847.93, 710.2, 596.67, 359.1, 347.1, 298.0, 392.0, 717.44, 697.0, 571.0, 530.92, 457.53, 390.63, 4224.44, 4851.32, 3604.41, 4047.54, 2876.77, 2375.89, 3102.12, 2820.47, 2794.0, 3150.95, 2756.0, 2702.57, 2626.59, 2548.62, 2468.69, 2386.8], dtype=np.float32)

NCORES = 8


class Cfg:
    def __init__(self, n_nodes, e_total, c_tot):
        self.N = n_nodes
        self.W = n_nodes // NCORES          # nodes owned per core
        self.NODE_F = math.ceil(n_nodes / 128 / 4) * 4   # free cols, mult of 4
        self.NPAD = 128 * self.NODE_F
        assert self.NPAD % 512 == 0
        self.NCHUNK = self.NPAD // 512
        self.QBINS = math.ceil(self.W / 128)
        self.BINS = 128 * self.QBINS
        self.C_TOT = c_tot                   # edge columns per core
        self.EPAD = 128 * c_tot
        self.F = min(512, c_tot)             # columns per edge tile


FULL = Cfg(100000, 6400000, 6320)

# folded constants
_PB = 2.0 * 2.54 * BOHR          # p * BOHR = _PB * alpha_ij^{1/7}
_C6F = C_FACTOR * HARTREE * BOHR ** 6
_B1 = math.log(FINE_STRUCTURE ** (-4.0 / 21.0)) - math.log(2.0) / 7.0
_B6 = 6.0 * math.log(_PB) - 6.0 * math.log(2.0) / 7.0
_B8 = 8.0 * math.log(_PB) - 8.0 * math.log(2.0) / 7.0
_B10 = 10.0 * math.log(_PB) - 10.0 * math.log(2.0) / 7.0
_GB0, _GB1, _GB2, _GB3 = -0.00433008, 0.24428889, 0.04125273, -0.00078893


def build_nc_node(cfg: Cfg):
    nc = bacc.Bacc("TRN2")
    h_nat = nc.dram_tensor("h_nat", [128, cfg.NODE_F], F32, kind="ExternalInput")
    z_cols = nc.dram_tensor("z_cols", [cfg.NPAD], F32, kind="ExternalInput")
    ac_tab = nc.dram_tensor("ac_tab", [128, 2], F32, kind="ExternalInput")
    iota_col = nc.dram_tensor("iota_col", [128, 1], F32, kind="ExternalInput")
    table = nc.dram_tensor("table_out", [cfg.NPAD, 2], F32, kind="ExternalOutput")

    # ---------------- node phase ----------------
    with tile.TileContext(nc) as tc, ExitStack() as ctx:
        consts = ctx.enter_context(tc.tile_pool(name="nconsts", bufs=1))
        pool = ctx.enter_context(tc.tile_pool(name="npool", bufs=3))
        psum = ctx.enter_context(tc.tile_pool(name="npsum", bufs=3, space="PSUM"))
        big = ctx.enter_context(tc.tile_pool(name="nbig", bufs=1))

        ic = consts.tile([128, 1], F32)
        nc.sync.dma_start(ic[:], iota_col[:])
        act = consts.tile([128, 2], F32)
        nc.sync.dma_start(act[:], ac_tab[:])
        hn = big.tile([128, cfg.NODE_F], F32, name="hn", tag="hn")
        nc.sync.dma_start(hn[:], h_nat[:])

        acn = big.tile([128, cfg.NODE_F, 2], F32, name="acn", tag="acn")
        for c in range(cfg.NCHUNK):
            zb = pool.tile([128, 512], F32, name="zb", tag="zb")
            nc.sync.dma_start(
                zb[:], z_cols[None, 512 * c:512 * (c + 1)].to_broadcast([128, 512]))
            oh = pool.tile([128, 512], F32, name="oh", tag="oh")
            nc.vector.tensor_tensor(
                out=oh[:], in0=zb[:], in1=ic[:].to_broadcast([128, 512]),
                op=mybir.AluOpType.is_equal)
            ps = psum.tile([128, 4, 2], F32, name="ps", tag="ps")
            for j in range(4):
                nc.tensor.matmul(ps[:, j, :],
                                 lhsT=oh[:, 128 * j:128 * (j + 1)],
                                 rhs=act[:], start=True, stop=True)
            nc.vector.tensor_copy(
                out=acn[:, 4 * c:4 * c + 4, :], in_=ps[:, :, :])
        # alpha = A*h ; C6 = C*h^2
        h2 = big.tile([128, cfg.NODE_F], F32, name="h2", tag="h2")
        nc.vector.tensor_mul(out=h2[:], in0=hn[:], in1=hn[:])
        nc.vector.tensor_mul(out=acn[:, :, 0], in0=acn[:, :, 0], in1=hn[:])
        nc.vector.tensor_mul(out=acn[:, :, 1], in0=acn[:, :, 1], in1=h2[:])
        nc.sync.dma_start(
            table.rearrange("(p f) c -> p f c", p=128), acn[:, :, :])
    nc.compile()
    return nc


def build_nc_edge(cfg: Cfg):
    nc = bacc.Bacc("TRN2")
    F = cfg.F
    n_tiles = (cfg.C_TOT + F - 1) // F
    table = nc.dram_tensor("table", [cfg.NPAD, 2], F32, kind="ExternalInput")
    n_gt = (cfg.C_TOT + 31) // 32
    wcols = sum(min(32, cfg.C_TOT - 32 * g) * 8 for g in range(n_gt))
    sblk = nc.dram_tensor("sblk", [128, wcols], mybir.dt.int16, kind="ExternalInput")
    rblk = nc.dram_tensor("rblk", [128, wcols], mybir.dt.int16, kind="ExternalInput")
    slo = nc.dram_tensor("slo", [128, cfg.C_TOT], F32, kind="ExternalInput")
    rlo = nc.dram_tensor("rlo", [128, cfg.C_TOT], F32, kind="ExternalInput")
    iota32 = nc.dram_tensor("iota32", [128, 32], F32, kind="ExternalInput")
    lens = nc.dram_tensor("lens", [128, cfg.C_TOT], F32, kind="ExternalInput")
    m_f = nc.dram_tensor("m_f", [128, cfg.C_TOT], F32, kind="ExternalInput")
    q_f = nc.dram_tensor("q_f", [128, cfg.C_TOT], F32, kind="ExternalInput")
    iota_r = nc.dram_tensor("iota_r", [128, 128], F32, kind="ExternalInput")
    iota_q = nc.dram_tensor("iota_q", [128, cfg.QBINS], F32, kind="ExternalInput")
    ident = nc.dram_tensor("ident", [128, 128], F32, kind="ExternalInput")
    ebias = nc.dram_tensor("ebias", [128, 4], F32, kind="ExternalInput")
    out = nc.dram_tensor("out", [cfg.QBINS, 128], F32, kind="ExternalOutput")
    sv_all = nc.dram_tensor("sv_all", [128, cfg.C_TOT, 2], F32, kind="Internal")
    rv_all = nc.dram_tensor("rv_all", [128, cfg.C_TOT, 2], F32, kind="Internal")

    # ------------- raw gather section (dma_gather block-32 + select) ----
    from concourse.library_config import mlp as _mlp_lib
    table_v = table.rearrange("(b w) c -> b (w c)", w=32)
    with ExitStack() as rctx:
        sbw = [rctx.enter_context(nc.sbuf_tensor(f"sbw{j}", [128, 32 * 8], mybir.dt.int16)) for j in range(2)]
        rbw = [rctx.enter_context(nc.sbuf_tensor(f"rbw{j}", [128, 32 * 8], mybir.dt.int16)) for j in range(2)]
        i32t = rctx.enter_context(nc.sbuf_tensor("i32t", [128, 32], F32))
        slot = [rctx.enter_context(nc.sbuf_tensor(f"slot{j}", [128, 32], F32)) for j in range(2)]
        rlot = [rctx.enter_context(nc.sbuf_tensor(f"rlot{j}", [128, 32], F32)) for j in range(2)]
        sg = [rctx.enter_context(nc.sbuf_tensor(f"sg{j}", [128, 32, 64], F32)) for j in range(2)]
        rg = [rctx.enter_context(nc.sbuf_tensor(f"rg{j}", [128, 32, 64], F32)) for j in range(2)]
        oh = [rctx.enter_context(nc.sbuf_tensor(f"oh{j}", [128, 32, 32], F32)) for j in range(2)]
        mm = [rctx.enter_context(nc.sbuf_tensor(f"mm{j}", [128, 32, 32], F32)) for j in range(2)]
        svr = [rctx.enter_context(nc.sbuf_tensor(f"svr{j}", [128, 32, 2], F32)) for j in range(2)]
        rvr = [rctx.enter_context(nc.sbuf_tensor(f"rvr{j}", [128, 32, 2], F32)) for j in range(2)]
        ld = rctx.enter_context(nc.semaphore("g_ld"))
        gs = rctx.enter_context(nc.semaphore("g_gs"))
        vs = rctx.enter_context(nc.semaphore("g_vs"))
        so = rctx.enter_context(nc.semaphore("g_so"))
        nc.gpsimd.load_library(_mlp_lib)
        dvec = [0]

        def dve_wait():
            if dvec[0]:
                nc.vector.wait_ge(vs, dvec[0])

        def dve_done(inst):
            inst.then_inc(vs, 1)
            dvec[0] += 1
        nc.gpsimd.dma_start(i32t.ap()[:, :], iota32[:, :]).then_inc(ld, 16)
        nc.gpsimd.wait_ge(ld, 16)
        ldc = 16
        wc0 = 0
        TT = mybir.AluOpType
        for g in range(n_gt):
            j = g % 2
            c0 = 32 * g
            fc = min(32, cfg.C_TOT - c0)
            ni = fc * 128
            if g >= 2:
                nc.gpsimd.wait_ge(so, 32 * (g - 1))
            nc.gpsimd.dma_start(slot[j].ap()[:, :fc], slo[:, c0:c0 + fc]).then_inc(ld, 16)
            nc.gpsimd.dma_start(rlot[j].ap()[:, :fc], rlo[:, c0:c0 + fc]).then_inc(ld, 16)
            nc.gpsimd.dma_start(sbw[j].ap()[:, :fc * 8], sblk[:, wc0:wc0 + fc * 8]).then_inc(ld, 16)
            nc.gpsimd.dma_start(rbw[j].ap()[:, :fc * 8], rblk[:, wc0:wc0 + fc * 8]).then_inc(ld, 16)
            ldc += 64
            nc.gpsimd.wait_ge(ld, ldc)
            nc.gpsimd.dma_gather(
                sg[j].ap()[:, :fc, :], table_v[:, :], sbw[j].ap()[:, :fc * 8],
                ni, ni, 64, single_packet=False).then_inc(gs, 16)
            nc.gpsimd.dma_gather(
                rg[j].ap()[:, :fc, :], table_v[:, :], rbw[j].ap()[:, :fc * 8],
                ni, ni, 64, single_packet=False).then_inc(gs, 16)
            wc0 += fc * 8
            nc.vector.wait_ge(gs, 32 * (g + 1))
            nc.vector.wait_ge(ld, ldc)
            # sender select
            dve_wait()
            _i = nc.vector.tensor_tensor(
                out=oh[j].ap()[:, :fc, :],
                in0=slot[j].ap()[:, :fc].unsqueeze(2).to_broadcast([128, fc, 32]),
                in1=i32t.ap()[:, :].unsqueeze(1).to_broadcast([128, fc, 32]),
                op=TT.is_equal)
            dve_done(_i)
            dve_wait()
            _i = nc.vector.tensor_tensor(
                out=mm[j].ap()[:, :fc, :], in0=oh[j].ap()[:, :fc, :],
                in1=sg[j].ap()[:, :fc, 0::2], op=TT.mult)
            dve_done(_i)
            dve_wait()
            _i = nc.vector.reduce_sum(svr[j].ap()[:, :fc, 0:1], mm[j].ap()[:, :fc, :],
                                 axis=mybir.AxisListType.X)
            dve_done(_i)
            dve_wait()
            _i = nc.vector.tensor_tensor(
                out=mm[j].ap()[:, :fc, :], in0=oh[j].ap()[:, :fc, :],
                in1=sg[j].ap()[:, :fc, 1::2], op=TT.mult)
            dve_done(_i)
            dve_wait()
            _i = nc.vector.reduce_sum(svr[j].ap()[:, :fc, 1:2], mm[j].ap()[:, :fc, :],
                                 axis=mybir.AxisListType.X)
            dve_done(_i)
            # receiver select
            dve_wait()
            _i = nc.vector.tensor_tensor(
                out=oh[j].ap()[:, :fc, :],
                in0=rlot[j].ap()[:, :fc].unsqueeze(2).to_broadcast([128, fc, 32]),
                in1=i32t.ap()[:, :].unsqueeze(1).to_broadcast([128, fc, 32]),
                op=TT.is_equal)
            dve_done(_i)
            dve_wait()
            _i = nc.vector.tensor_tensor(
                out=mm[j].ap()[:, :fc, :], in0=oh[j].ap()[:, :fc, :],
                in1=rg[j].ap()[:, :fc, 0::2], op=TT.mult)
            dve_done(_i)
            dve_wait()
            _i = nc.vector.reduce_sum(rvr[j].ap()[:, :fc, 0:1], mm[j].ap()[:, :fc, :],
                                 axis=mybir.AxisListType.X)
            dve_done(_i)
            dve_wait()
            _i = nc.vector.tensor_tensor(
                out=mm[j].ap()[:, :fc, :], in0=oh[j].ap()[:, :fc, :],
                in1=rg[j].ap()[:, :fc, 1::2], op=TT.mult)
            dve_done(_i)
            dve_wait()
            _i = nc.vector.reduce_sum(
                rvr[j].ap()[:, :fc, 1:2], mm[j].ap()[:, :fc, :],
                axis=mybir.AxisListType.X)
            dve_done(_i)
            nc.gpsimd.wait_ge(vs, dvec[0])
            nc.gpsimd.dma_start(sv_all[:, c0:c0 + fc, :], svr[j].ap()[:, :fc, :]).then_inc(so, 16)
            nc.gpsimd.dma_start(rv_all[:, c0:c0 + fc, :], rvr[j].ap()[:, :fc, :]).then_inc(so, 16)
        nc.gpsimd.wait_ge(so, 32 * n_gt)
    nc.all_engine_barrier()

    # ---------------- edge phase ----------------
    with tile.TileContext(nc) as tc, ExitStack() as ctx:
        consts = ctx.enter_context(tc.tile_pool(name="econsts", bufs=1))
        inp = ctx.enter_context(tc.tile_pool(name="einp", bufs=2))
        gat = ctx.enter_context(tc.tile_pool(name="egat", bufs=2))
        tmp = ctx.enter_context(tc.tile_pool(name="etmp", bufs=1))
        ohp = ctx.enter_context(tc.tile_pool(name="eoh", bufs=1))
        psum = ctx.enter_context(tc.tile_pool(name="epsum", bufs=1, space="PSUM"))
        psum2 = ctx.enter_context(tc.tile_pool(name="epsum2", bufs=1, space="PSUM"))

        ir = consts.tile([128, 128], F32)
        nc.sync.dma_start(ir[:], iota_r[:])
        iq = consts.tile([128, cfg.QBINS], F32)
        nc.sync.dma_start(iq[:], iota_q[:])
        idn = consts.tile([128, 128], F32)
        nc.sync.dma_start(idn[:], ident[:])
        eb = consts.tile([128, 4], F32)
        nc.sync.dma_start(eb[:], ebias[:])

        bins = psum.tile([128, cfg.QBINS], F32)

        TT = mybir.AluOpType
        AF = mybir.ActivationFunctionType
        n_mm = 0
        total_mm = cfg.C_TOT

        for t in range(n_tiles):
            c0 = t * F
            f = min(F, cfg.C_TOT - c0)
            lt = inp.tile([128, F], F32, name="lt", tag="lt")
            nc.sync.dma_start(lt[:, :f], lens[:, c0:c0 + f])
            mf = inp.tile([128, F], F32, name="mf", tag="mf")
            nc.sync.dma_start(mf[:, :f], m_f[:, c0:c0 + f])
            qf = inp.tile([128, F], F32, name="qf", tag="qf")
            nc.sync.dma_start(qf[:, :f], q_f[:, c0:c0 + f])

            sv = gat.tile([128, F, 2], F32, name="sv", tag="sv")
            nc.sync.dma_start(sv[:, :f, :], sv_all[:, c0:c0 + f, :])
            rv = gat.tile([128, F, 2], F32, name="rv", tag="rv")
            nc.sync.dma_start(rv[:, :f, :], rv_all[:, c0:c0 + f, :])

            als = sv[:, :f, 0]
            cs = sv[:, :f, 1]
            alr = rv[:, :f, 0]
            cr = rv[:, :f, 1]

            def T(tag):
                return tmp.tile([128, F], F32, name=tag, tag=tag)[:, :f]

            a2 = T("a2"); nc.vector.tensor_add(out=a2, in0=als, in1=alr)
            u = T("u"); nc.vector.tensor_mul(out=u, in0=alr, in1=cs)
            tv = T("tv"); nc.vector.tensor_mul(out=tv, in0=als, in1=cr)
            ut = T("ut"); nc.vector.tensor_mul(out=ut, in0=u, in1=tv)
            du = T("du"); nc.vector.tensor_mul(out=du, in0=alr, in1=u)
            dt = T("dt"); nc.vector.tensor_mul(out=dt, in0=als, in1=tv)
            den = T("den"); nc.vector.tensor_add(out=den, in0=du, in1=dt)
            rden = T("rden"); nc.vector.reciprocal(out=rden, in_=den)
            c6p = T("c6p"); nc.vector.tensor_mul(out=c6p, in0=ut, in1=rden)

            la = T("la"); nc.scalar.activation(out=la, in_=a2, func=AF.Ln)
            q1 = T("q1"); nc.scalar.activation(out=q1, in_=la, func=AF.Exp,
                                               scale=1.0 / 7.0, bias=eb[:, 0:1])
            p6 = T("p6"); nc.scalar.activation(out=p6, in_=la, func=AF.Exp,
                                               scale=6.0 / 7.0, bias=eb[:, 1:2])
            p8 = T("p8"); nc.scalar.activation(out=p8, in_=la, func=AF.Exp,
                                               scale=8.0 / 7.0, bias=eb[:, 2:3])
            p10 = T("p10"); nc.scalar.activation(out=p10, in_=la, func=AF.Exp,
                                                 scale=10.0 / 7.0, bias=eb[:, 3:4])
            # s = b3 v^3 + b2 v^2 + b1 v + b0  (Horner)
            hh = T("hh"); nc.scalar.activation(out=hh, in_=q1, func=AF.Copy,
                                               scale=_GB3, bias=_GB2)
            h3 = T("h3"); nc.vector.tensor_mul(out=h3, in0=hh, in1=q1)
            nc.vector.tensor_scalar_add(out=h3, in0=h3, scalar1=_GB1)
            sres = T("sres"); nc.vector.tensor_mul(out=sres, in0=h3, in1=q1)
            nc.vector.tensor_scalar_add(out=sres, in0=sres, scalar1=_GB0)
            s2 = T("s2"); nc.vector.tensor_mul(out=s2, in0=sres, in1=sres)
            s4 = T("s4"); nc.vector.tensor_mul(out=s4, in0=s2, in1=s2)
            nc.vector.tensor_scalar_mul(out=s2, in0=s2, scalar1=10.0 * BOHR ** 2)
            nc.vector.tensor_scalar_mul(out=s4, in0=s4, scalar1=122.5 * BOHR ** 4)

            l2 = T("l2"); nc.vector.tensor_mul(out=l2, in0=lt[:, :f], in1=lt[:, :f])
            l4 = T("l4"); nc.vector.tensor_mul(out=l4, in0=l2, in1=l2)
            l6 = T("l6"); nc.vector.tensor_mul(out=l6, in0=l4, in1=l2)
            l8 = T("l8"); nc.vector.tensor_mul(out=l8, in0=l4, in1=l4)
            l10 = T("l10"); nc.vector.tensor_mul(out=l10, in0=l6, in1=l4)
            nc.vector.tensor_add(out=l6, in0=l6, in1=p6)
            nc.vector.tensor_add(out=l8, in0=l8, in1=p8)
            nc.vector.tensor_add(out=l10, in0=l10, in1=p10)
            r6 = T("r6"); nc.vector.reciprocal(out=r6, in_=l6)
            r8 = T("r8"); nc.vector.reciprocal(out=r8, in_=l8)
            r10 = T("r10"); nc.vector.reciprocal(out=r10, in_=l10)
            m8 = T("m8"); nc.vector.tensor_mul(out=m8, in0=s2, in1=r8)
            m10 = T("m10"); nc.vector.tensor_mul(out=m10, in0=s4, in1=r10)
            nc.vector.tensor_add(out=r6, in0=r6, in1=m8)
            nc.vector.tensor_add(out=r6, in0=r6, in1=m10)
            epre = T("epre"); nc.vector.tensor_mul(out=epre, in0=c6p, in1=r6)
            nc.vector.tensor_scalar_mul(out=epre, in0=epre, scalar1=-2.0 * _C6F)

            # switching function
            cx = T("cx"); nc.scalar.activation(out=cx, in_=lt[:, :f], func=AF.Copy,
                                               scale=0.5, bias=-4.0)
            x1 = T("x1"); nc.scalar.activation(out=x1, in_=cx, func=AF.Copy,
                                               scale=-1.0, bias=1.0)
            nc.vector.tensor_scalar_max(out=x1, in0=x1, scalar1=1e-12)
            x2 = T("x2"); nc.vector.tensor_scalar_max(out=x2, in0=cx, scalar1=1e-12)
            n1 = T("n1"); nc.vector.reciprocal(out=n1, in_=x1)
            n2 = T("n2"); nc.vector.reciprocal(out=n2, in_=x2)
            nc.vector.tensor_scalar_min(out=n1, in0=n1, scalar1=87.0)
            nc.vector.tensor_scalar_min(out=n2, in0=n2, scalar1=87.0)
            e1 = T("e1"); nc.scalar.activation(out=e1, in_=n1, func=AF.Exp, scale=-1.0)
            e2 = T("e2"); nc.scalar.activation(out=e2, in_=n2, func=AF.Exp, scale=-1.0)
            ws = T("ws"); nc.vector.tensor_add(out=ws, in0=e1, in1=e2)
            nc.vector.tensor_scalar_add(out=ws, in0=ws, scalar1=1e-12)
            rw = T("rw"); nc.vector.reciprocal(out=rw, in_=ws)
            wv = T("wv"); nc.vector.tensor_mul(out=wv, in0=e1, in1=rw)
            v = T("v"); nc.vector.tensor_mul(out=v, in0=epre, in1=wv)


            # scatter: one-hot matmuls, batches of 32 columns
            BW = 32
            for b0 in range(0, f, BW):
                bw = min(BW, f - b0)
                ohr = ohp.tile([128, BW, 128], F32, name="ohr", tag="ohr")
                nc.vector.tensor_tensor(
                    out=ohr[:, :bw, :],
                    in0=mf[:, b0:b0 + bw].unsqueeze(2).to_broadcast([128, bw, 128]),
                    in1=ir[:].unsqueeze(1).to_broadcast([128, bw, 128]),
                    op=TT.is_equal)
                ohq = ohp.tile([128, BW, cfg.QBINS], F32, name="ohq", tag="ohq")
                nc.vector.tensor_tensor(
                    out=ohq[:, :bw, :],
                    in0=qf[:, b0:b0 + bw].unsqueeze(2).to_broadcast(
                        [128, bw, cfg.QBINS]),
                    in1=iq[:].unsqueeze(1).to_broadcast([128, bw, cfg.QBINS]),
                    op=TT.is_equal)
                nc.vector.tensor_tensor(
                    out=ohq[:, :bw, :],
                    in0=ohq[:, :bw, :],
                    in1=v[:, b0:b0 + bw].unsqueeze(2).to_broadcast(
                        [128, bw, cfg.QBINS]),
                    op=TT.mult)
                for j in range(bw):
                    nc.tensor.matmul(
                        bins[:, :], lhsT=ohr[:, j, :], rhs=ohq[:, j, :],
                        start=(n_mm == 0), stop=(n_mm == total_mm - 1))
                    n_mm += 1

        # transpose bins [128, QBINS] -> [QBINS, 128] and write out
        bsb = consts.tile([128, cfg.QBINS], F32)
        nc.vector.tensor_copy(out=bsb[:], in_=bins[:])
        btp = psum2.tile([128, 128], F32)
        nc.tensor.transpose(out=btp[:cfg.QBINS, :], in_=bsb[:], identity=idn[:])
        bts = consts.tile([cfg.QBINS, 128], F32)
        nc.vector.tensor_copy(out=bts[:], in_=btp[:cfg.QBINS, :])
        nc.sync.dma_start(out[:, :], bts[:])

    nc.compile()
    return nc


_NC_CACHE = {}


def _get_nc(cfg, which):
    key = (cfg.N, cfg.C_TOT, which)
    if key not in _NC_CACHE:
        _NC_CACHE[key] = (build_nc_node(cfg) if which == "node"
                          else build_nc_edge(cfg))
    return _NC_CACHE[key]


def shard_inputs(cfg, hirshfeld_ratios, atomic_numbers, senders_lr, receivers_lr,
                 lengths_lr):
    N, W, EPAD = cfg.N, cfg.W, cfg.EPAD
    h = np.asarray(hirshfeld_ratios, np.float32)
    z = np.asarray(atomic_numbers, np.int32)
    s = np.asarray(senders_lr, np.int32)
    r = np.asarray(receivers_lr, np.int32)
    ln = np.asarray(lengths_lr, np.float32)

    hp = np.ones(cfg.NPAD, np.float32)
    hp[:N] = h
    zp = np.ones(cfg.NPAD, np.int32)
    zp[:N] = z
    h_nat = hp.reshape(128, cfg.NODE_F)
    z_cols = (zp.astype(np.float32) - 1.0).reshape(128, cfg.NODE_F).T.copy().reshape(-1)
    ac_tab = np.zeros((128, 2), np.float32)
    ac_tab[:len(ALPHAS), 0] = ALPHAS
    ac_tab[:len(C6_COEF), 1] = C6_COEF

    iota_col = np.arange(128, dtype=np.float32).reshape(128, 1)
    iota_r = np.tile(np.arange(128, dtype=np.float32), (128, 1))
    iota_q = np.tile(np.arange(cfg.QBINS, dtype=np.float32), (128, 1))
    ident = np.eye(128, dtype=np.float32)

    core_of = r // W
    order = np.argsort(core_of, kind="stable")
    s_o, r_o, l_o, c_o = s[order], r[order], ln[order], core_of[order]
    bounds = np.searchsorted(c_o, np.arange(NCORES + 1))

    in_maps = []
    for c in range(NCORES):
        lo, hi = bounds[c], bounds[c + 1]
        cnt = hi - lo
        assert cnt <= EPAD, f"core {c} edge count {cnt} > EPAD {EPAD}"
        base = c * W
        sp = np.zeros(EPAD, np.int32)
        rp = np.full(EPAD, base, np.int32)
        lp = np.full(EPAD, 100.0, np.float32)
        sp[:cnt] = s_o[lo:hi]
        rp[:cnt] = r_o[lo:hi]
        lp[:cnt] = l_o[lo:hi]
        rloc = rp - base
        mfv = (rloc & 127).astype(np.float32)
        qfv = (rloc >> 7).astype(np.float32)

        def wrap_blk(arr):
            blk2 = (arr >> 5).astype(np.int16).reshape(128, cfg.C_TOT)
            parts = []
            n_gt = (cfg.C_TOT + 31) // 32
            for g in range(n_gt):
                c0 = 32 * g
                fc = min(32, cfg.C_TOT - c0)
                unw = blk2[:, c0:c0 + fc].T.reshape(-1)       # i = c*128+p
                w16 = unw.reshape(fc * 8, 16).T               # [16, fc*8]
                parts.append(np.tile(w16, (8, 1)))
            return np.concatenate(parts, axis=1)
        in_maps.append({
            "sblk": wrap_blk(sp), "rblk": wrap_blk(rp),
            "slo": (sp & 31).astype(np.float32).reshape(128, cfg.C_TOT),
            "rlo": (rp & 31).astype(np.float32).reshape(128, cfg.C_TOT),
            "iota32": np.tile(np.arange(32, dtype=np.float32), (128, 1)),
            "lens": lp.reshape(128, cfg.C_TOT),
            "m_f": mfv.reshape(128, cfg.C_TOT),
            "q_f": qfv.reshape(128, cfg.C_TOT),
            "iota_r": iota_r, "iota_q": iota_q,
            "ident": ident,
            "ebias": np.tile(np.array([[_B1, _B6, _B8, _B10]], np.float32), (128, 1)),
        })
    node_map = {"h_nat": h_nat, "z_cols": z_cols, "ac_tab": ac_tab,
                "iota_col": iota_col}
    return node_map, in_maps


def unshard(cfg, results):
    outp = np.zeros(cfg.N, np.float32)
    for c in range(NCORES):
        o = results[c]["out"].reshape(-1)[:cfg.W]
        outp[c * cfg.W:(c + 1) * cfg.W] = o
    return outp.reshape(-1, 1)


def run_all(cfg, node_map, in_maps):
    nc_node = _get_nc(cfg, "node")
    nc_edge = _get_nc(cfg, "edge")
    resn = run_bass_kernel_spmd(nc_node, [node_map], core_ids=[0])
    table = resn.results[0]["table_out"]
    for im in in_maps:
        im["table"] = table
    res = run_bass_kernel_spmd(nc_edge, in_maps, core_ids=list(range(NCORES)))
    return res


def kernel(hirshfeld_ratios, atomic_numbers, senders_lr, receivers_lr,
           lengths_lr, num_nodes):
    cfg = FULL
    assert int(num_nodes) == cfg.N
    node_map, in_maps = shard_inputs(cfg, hirshfeld_ratios, atomic_numbers,
                                     senders_lr, receivers_lr, lengths_lr)
    res = run_all(cfg, node_map, in_maps)
    return unshard(cfg, res.results)



# revision 2
# speedup vs baseline: 5.9268x; 5.9268x over previous
"""Trainium2 Bass kernel for nn_DispersionInteraction (vdW-QDO dispersion).

Strategy (8 NeuronCores, SPMD single NEFF):
  - Edges are sharded across cores by RECEIVER block (core c owns nodes
    [c*12500, (c+1)*12500)), so each core's local segment-sum covers only
    12544 bins and no cross-core reduction is needed (outputs concatenate).
  - Single NEFF per core, four phases:
      1. Node phase (Tile): builds the (alpha_n, C6_n) = (A[z]*h, C[z]*h^2)
         table on-device via one-hot matmul on the tensor engine, into a raw
         SBUF buffer.
      2. Prep phase (raw): writes the table to Internal DRAM, replicates the
         [16, E/16] wrapped gather indices to the 128-partition layout
         dma_gather needs, expands int8 sideband inputs (slo, rlo derived
         from m) to f32 selection keys, and generates iota/identity tensors
         on-device (gpsimd iota) so none of them are uploaded.
      3. Gather phase (raw): per-edge (alpha, C6) records for sender and
         receiver fetched with the GPSIMD dma_gather ucode op at 32-node
         block granularity (256B rows, int16 block ids), then the right 8B
         record selected on the vector engine with a one-hot over the low 5
         index bits.
      4. Edge phase (Tile): per-edge energies via DVE/ACT ops; segment-sum
         on the tensor engine via one-hot matmuls accumulating into a PSUM
         [128, 98] bin grid (bin = (r_local & 127, r_local >> 7)).
  - Host->device payload is minimized (~9B/edge vs 52B/edge naive): block
    ids as non-replicated int16, 5/7-bit sidebands as int8, lengths as fp16.
    The axon PJRT tunnel is ~40-70 MB/s, so upload bytes dominate wall time.
  - Execution uses a cached jit of the shard_map'd bass_exec call, so
    repeated runs don't re-trace or re-lower.
"""

import math
import sys

import numpy as np

sys.path.insert(0, "/opt/trn_rl_repo")

import jax
from jax.experimental.shard_map import shard_map
from jax.sharding import Mesh, PartitionSpec

import concourse.bass as bass
import concourse.tile as tile
from concourse import bacc, mybir
from contextlib import ExitStack

F32 = mybir.dt.float32
F16 = mybir.dt.float16
I32 = mybir.dt.int32
I16 = mybir.dt.int16
I8 = mybir.dt.int8

BOHR = 0.5291772105638411
FINE_STRUCTURE = 0.0072973525693
HARTREE = 27.211386245988
C_FACTOR = 0.5

ALPHAS = np.array([4.5, 1.38, 164.2, 38.0, 21.0, 12.0, 7.4, 5.4, 3.8, 2.67, 162.7, 71.0, 60.0, 37.0, 25.0, 19.6, 15.0, 11.1, 292.9, 160.0, 120.0, 98.0, 84.0, 78.0, 63.0, 56.0, 50.0, 48.0, 42.0, 40.0, 60.0, 41.0, 29.0, 25.0, 20.0, 16.8, 319.2, 199.0, 126.74, 119.97, 101.6, 88.42, 80.08, 65.89, 56.1, 23.68, 50.6, 39.7, 70.22, 55.95, 43.67, 37.65, 35.0, 27.3, 399.9, 275.0, 213.7, 204.7, 215.8, 208.4, 200.2, 192.1, 184.2, 158.3, 169.5, 164.64, 156.3, 150.2, 144.3, 138.9, 137.2, 99.52, 82.53, 71.04, 63.04, 55.06, 42.51, 39.68, 36.5, 33.9, 69.92, 61.8, 49.02, 45.01, 38.93, 33.54, 317.8, 246.2, 203.3, 217.0, 154.4, 127.8, 150.5, 132.2, 131.2, 143.6, 125.3, 121.5, 117.5, 113.4, 109.4, 105.4], dtype=np.float32)
C6_COEF = np.array([6.5, 1.46, 1387.0, 214.0, 99.5, 46.6, 24.2, 15.6, 9.52, 6.38, 1556.0, 627.0, 528.0, 305.0, 185.0, 134.0, 94.6, 64.3, 3897.0, 2221.0, 1383.0, 1044.0, 832.0, 602.0, 552.0, 482.0, 408.0, 373.0, 253.0, 284.0, 498.0, 354.0, 246.0, 210.0, 162.0, 129.6, 4691.0, 3170.0, 1968.58, 1677.91, 1263.61, 1028.73, 1390.87, 609.75, 469.0, 157.5, 339.0, 452.0, 707.05, 587.42, 459.32, 396.0, 385.0, 285.9, 6846.0, 5727.0, 3884.5, 3708.33, 3911.84, 3908.75, 3847.68, 3708.69, 3511.71, 2781.53, 3124.41, 2984.29, 2839.95, 2724.12, 2576.78, 2387.53, 2371.8, 1274.8, 1019.92, 847.93, 710.2, 596.67, 359.1, 347.1, 298.0, 392.0, 717.44, 697.0, 571.0, 530.92, 457.53, 390.63, 4224.44, 4851.32, 3604.41, 4047.54, 2876.77, 2375.89, 3102.12, 2820.47, 2794.0, 3150.95, 2756.0, 2702.57, 2626.59, 2548.62, 2468.69, 2386.8], dtype=np.float32)

NCORES = 8


class Cfg:
    def __init__(self, n_nodes, e_total, c_tot):
        self.N = n_nodes
        self.W = n_nodes // NCORES          # nodes owned per core
        self.NODE_F = math.ceil(n_nodes / 128 / 4) * 4   # free cols, mult of 4
        self.NPAD = 128 * self.NODE_F
        assert self.NPAD % 512 == 0
        self.NCHUNK = self.NPAD // 512
        self.QBINS = math.ceil(self.W / 128)
        self.BINS = 128 * self.QBINS
        self.C_TOT = c_tot                   # edge columns per core
        self.EPAD = 128 * c_tot
        self.F = min(512, c_tot)             # columns per edge tile
        self.N_GT = (c_tot + 31) // 32       # gather groups of 32 cols
        self.WC = c_tot * 8                  # wrapped idx cols


FULL = Cfg(100000, 6400000, 6320)

# folded constants
_PB = 2.0 * 2.54 * BOHR          # p * BOHR = _PB * alpha_ij^{1/7}
_C6F = C_FACTOR * HARTREE * BOHR ** 6
_B1 = math.log(FINE_STRUCTURE ** (-4.0 / 21.0)) - math.log(2.0) / 7.0
_B6 = 6.0 * math.log(_PB) - 6.0 * math.log(2.0) / 7.0
_B8 = 8.0 * math.log(_PB) - 8.0 * math.log(2.0) / 7.0
_B10 = 10.0 * math.log(_PB) - 10.0 * math.log(2.0) / 7.0
_GB0, _GB1, _GB2, _GB3 = -0.00433008, 0.24428889, 0.04125273, -0.00078893


def build_nc(cfg: Cfg):
    nc = bacc.Bacc("TRN2")
    F = cfg.F
    n_tiles = (cfg.C_TOT + F - 1) // F

    # ---- inputs ----
    h16 = nc.dram_tensor("h16", [128, cfg.NODE_F], F16, kind="ExternalInput")
    z8d = nc.dram_tensor("z8", [cfg.NPAD], I8, kind="ExternalInput")
    ac_tab = nc.dram_tensor("ac_tab", [128, 2], F32, kind="ExternalInput")
    iota_col = nc.dram_tensor("iota_col", [128, 1], F32, kind="ExternalInput")
    sblk16 = nc.dram_tensor("sblk16", [16, cfg.WC], I16, kind="ExternalInput")
    rblk16 = nc.dram_tensor("rblk16", [16, cfg.WC], I16, kind="ExternalInput")
    slo8d = nc.dram_tensor("slo8", [128, cfg.C_TOT], I8, kind="ExternalInput")
    m8d = nc.dram_tensor("m8", [128, cfg.C_TOT], I8, kind="ExternalInput")
    q8d = nc.dram_tensor("q8", [128, cfg.C_TOT], I8, kind="ExternalInput")
    lens16 = nc.dram_tensor("lens16", [128, cfg.C_TOT], F16, kind="ExternalInput")
    b0sd = nc.dram_tensor("b0s", [128, 1], I32, kind="ExternalInput")
    ebias = nc.dram_tensor("ebias", [128, 4], F32, kind="ExternalInput")
    out = nc.dram_tensor("out", [cfg.QBINS, 128], F32, kind="ExternalOutput")

    # ---- internals ----
    table = nc.dram_tensor("table", [cfg.NPAD, 2], F32, kind="Internal")
    sblkR = nc.dram_tensor("sblkR", [128, cfg.WC], I16, kind="Internal")
    rblkR = nc.dram_tensor("rblkR", [128, cfg.WC], I16, kind="Internal")
    slo_f_d = nc.dram_tensor("slo_f_d", [128, cfg.C_TOT], F32, kind="Internal")
    rlo_f_d = nc.dram_tensor("rlo_f_d", [128, cfg.C_TOT], F32, kind="Internal")
    io32f_d = nc.dram_tensor("io32f_d", [128, 32], F32, kind="Internal")
    ir8_d = nc.dram_tensor("ir8_d", [128, 128], I8, kind="Internal")
    iq8_d = nc.dram_tensor("iq8_d", [128, cfg.QBINS], I8, kind="Internal")
    identf_d = nc.dram_tensor("identf_d", [128, 128], F32, kind="Internal")
    sv_all = nc.dram_tensor("sv_all", [128, cfg.C_TOT, 2], F32, kind="Internal")
    rv_all = nc.dram_tensor("rv_all", [128, cfg.C_TOT, 2], F32, kind="Internal")

    TT = mybir.AluOpType
    AF = mybir.ActivationFunctionType

    with ExitStack() as octx:
        # node table staging buffer, lives across the node tc + prep phase
        acn = octx.enter_context(
            nc.sbuf_tensor("acn_raw", [128, cfg.NODE_F, 2], F32))

        # ---------------- node phase (Tile) ----------------
        with tile.TileContext(nc) as tc, ExitStack() as ctx:
            consts = ctx.enter_context(tc.tile_pool(name="nconsts", bufs=1))
            pool = ctx.enter_context(tc.tile_pool(name="npool", bufs=3))
            psum = ctx.enter_context(tc.tile_pool(name="npsum", bufs=3,
                                                  space="PSUM"))
            ic = consts.tile([128, 1], F32)
            nc.sync.dma_start(ic[:], iota_col[:])
            act = consts.tile([128, 2], F32)
            nc.sync.dma_start(act[:], ac_tab[:])
            hn = consts.tile([128, cfg.NODE_F], F16)
            nc.sync.dma_start(hn[:], h16[:])
            hf = consts.tile([128, cfg.NODE_F], F32)
            nc.vector.tensor_copy(out=hf[:], in_=hn[:])

            for c in range(cfg.NCHUNK):
                zb8 = pool.tile([128, 512], I8, name="zb8", tag="zb8")
                nc.sync.dma_start(
                    zb8[:], z8d[None, 512 * c:512 * (c + 1)]
                    .to_broadcast([128, 512]))
                zbf = pool.tile([128, 512], F32, name="zbf", tag="zbf")
                nc.vector.tensor_copy(out=zbf[:], in_=zb8[:])
                oh = pool.tile([128, 512], F32, name="oh", tag="oh")
                nc.vector.tensor_tensor(
                    out=oh[:], in0=zbf[:], in1=ic[:].to_broadcast([128, 512]),
                    op=TT.is_equal)
                ps = psum.tile([128, 4, 2], F32, name="ps", tag="ps")
                for j in range(4):
                    nc.tensor.matmul(ps[:, j, :],
                                     lhsT=oh[:, 128 * j:128 * (j + 1)],
                                     rhs=act[:], start=True, stop=True)
                nc.vector.tensor_copy(
                    out=acn.ap()[:, 4 * c:4 * c + 4, :], in_=ps[:, :, :])
            # alpha = A*h ; C6 = C*h^2  (acn is raw SBUF; DVE is in-order)
            h2 = consts.tile([128, cfg.NODE_F], F32)
            nc.vector.tensor_mul(out=h2[:], in0=hf[:], in1=hf[:])
            nc.vector.tensor_mul(out=acn.ap()[:, :, 0],
                                 in0=acn.ap()[:, :, 0], in1=hf[:])
            nc.vector.tensor_mul(out=acn.ap()[:, :, 1],
                                 in0=acn.ap()[:, :, 1], in1=h2[:])
        nc.all_engine_barrier()

        # ---------------- prep + gather phase (raw) ----------------
        from concourse.library_config import mlp as _mlp_lib
        table_v = table.rearrange("(b w) c -> b (w c)", w=32)
        with ExitStack() as rctx:
            # prep tensors
            slo8s = rctx.enter_context(
                nc.sbuf_tensor("slo8s", [128, cfg.C_TOT], I8))
            m8s = rctx.enter_context(
                nc.sbuf_tensor("m8s", [128, cfg.C_TOT], I8))
            b0ss = rctx.enter_context(nc.sbuf_tensor("b0ss", [128, 1], I32))
            m32 = rctx.enter_context(
                nc.sbuf_tensor("m32", [128, cfg.C_TOT], I32))
            slo_f = rctx.enter_context(
                nc.sbuf_tensor("slo_f", [128, cfg.C_TOT], F32))
            rlo_f = rctx.enter_context(
                nc.sbuf_tensor("rlo_f", [128, cfg.C_TOT], F32))
            io32 = rctx.enter_context(nc.sbuf_tensor("io32", [128, 32], I32))
            io32f = rctx.enter_context(nc.sbuf_tensor("io32f", [128, 32], F32))
            ior32 = rctx.enter_context(nc.sbuf_tensor("ior32", [128, 128], I32))
            iorP = rctx.enter_context(nc.sbuf_tensor("iorP", [128, 128], I32))
            ioq32 = rctx.enter_context(
                nc.sbuf_tensor("ioq32", [128, cfg.QBINS], I32))
            ir8 = rctx.enter_context(nc.sbuf_tensor("ir8", [128, 128], I8))
            iq8 = rctx.enter_context(
                nc.sbuf_tensor("iq8", [128, cfg.QBINS], I8))
            identf = rctx.enter_context(
                nc.sbuf_tensor("identf", [128, 128], F32))

            tld = rctx.enter_context(nc.semaphore("tld"))
            tio = rctx.enter_context(nc.semaphore("tio"))
            tdv = rctx.enter_context(nc.semaphore("tdv"))
            tst = rctx.enter_context(nc.semaphore("tst"))

            # prep loads
            nc.sync.dma_start(slo8s.ap()[:, :], slo8d[:, :]).then_inc(tld, 16)
            nc.sync.dma_start(m8s.ap()[:, :], m8d[:, :]).then_inc(tld, 16)
            nc.sync.dma_start(b0ss.ap()[:, :], b0sd[:, :]).then_inc(tld, 16)

            # stores with no prep dependency: table + idx replicates
            nc.sync.dma_start(
                table.rearrange("(p f) c -> p f c", p=128),
                acn.ap()[:, :, :]).then_inc(tst, 16)
            nc.sync.dma_start(
                sblkR.rearrange("(a b) x -> a b x", a=8),
                sblk16[None, :, :].to_broadcast([8, 16, cfg.WC])
            ).then_inc(tst, 16)
            nc.sync.dma_start(
                rblkR.rearrange("(a b) x -> a b x", a=8),
                rblk16[None, :, :].to_broadcast([8, 16, cfg.WC])
            ).then_inc(tst, 16)

            # iotas on gpsimd
            nc.gpsimd.iota(io32.ap()[:, :], pattern=[[1, 32]], base=0,
                           channel_multiplier=0).then_inc(tio, 1)
            nc.gpsimd.iota(ior32.ap()[:, :], pattern=[[1, 128]], base=0,
                           channel_multiplier=0).then_inc(tio, 1)
            nc.gpsimd.iota(iorP.ap()[:, :], pattern=[[0, 128]], base=0,
                           channel_multiplier=1).then_inc(tio, 1)
            nc.gpsimd.iota(ioq32.ap()[:, :], pattern=[[1, cfg.QBINS]], base=0,
                           channel_multiplier=0).then_inc(tio, 1)

            # DVE prep chain (in order)
            nc.vector.wait_ge(tld, 48)
            nc.vector.wait_ge(tio, 4)
            nc.vector.tensor_copy(out=slo_f.ap()[:, :], in_=slo8s.ap()[:, :])
            nc.vector.tensor_copy(out=m32.ap()[:, :], in_=m8s.ap()[:, :])
            nc.vector.tensor_scalar(out=m32.ap()[:, :], in0=m32.ap()[:, :],
                                    scalar1=31, scalar2=None,
                                    op0=TT.bitwise_and)
            nc.vector.tensor_tensor(
                out=m32.ap()[:, :], in0=m32.ap()[:, :],
                in1=b0ss.ap()[:, 0:1].to_broadcast([128, cfg.C_TOT]),
                op=TT.add)
            nc.vector.tensor_scalar(out=m32.ap()[:, :], in0=m32.ap()[:, :],
                                    scalar1=31, scalar2=None,
                                    op0=TT.bitwise_and)
            nc.vector.tensor_copy(out=rlo_f.ap()[:, :], in_=m32.ap()[:, :])
            nc.vector.tensor_copy(out=io32f.ap()[:, :], in_=io32.ap()[:, :])
            nc.vector.tensor_copy(out=ir8.ap()[:, :], in_=ior32.ap()[:, :])
            nc.vector.tensor_copy(out=iq8.ap()[:, :], in_=ioq32.ap()[:, :])
            nc.vector.tensor_tensor(out=identf.ap()[:, :], in0=iorP.ap()[:, :],
                                    in1=ior32.ap()[:, :],
                                    op=TT.is_equal).then_inc(tdv, 1)

            # prep stores
            nc.sync.wait_ge(tdv, 1)
            nc.sync.dma_start(slo_f_d[:, :], slo_f.ap()[:, :]).then_inc(tst, 16)
            nc.sync.dma_start(rlo_f_d[:, :], rlo_f.ap()[:, :]).then_inc(tst, 16)
            nc.sync.dma_start(io32f_d[:, :], io32f.ap()[:, :]).then_inc(tst, 16)
            nc.sync.dma_start(ir8_d[:, :], ir8.ap()[:, :]).then_inc(tst, 16)
            nc.sync.dma_start(iq8_d[:, :], iq8.ap()[:, :]).then_inc(tst, 16)
            nc.sync.dma_start(identf_d[:, :], identf.ap()[:, :]
                              ).then_inc(tst, 16)

            # ------------- gather section (dma_gather block-32 + select) ----
            sbw = [rctx.enter_context(nc.sbuf_tensor(f"sbw{j}", [128, 32 * 8], I16)) for j in range(2)]
            rbw = [rctx.enter_context(nc.sbuf_tensor(f"rbw{j}", [128, 32 * 8], I16)) for j in range(2)]
            i32t = rctx.enter_context(nc.sbuf_tensor("i32t", [128, 32], F32))
            slot = [rctx.enter_context(nc.sbuf_tensor(f"slot{j}", [128, 32], F32)) for j in range(2)]
            rlot = [rctx.enter_context(nc.sbuf_tensor(f"rlot{j}", [128, 32], F32)) for j in range(2)]
            sg = [rctx.enter_context(nc.sbuf_tensor(f"sg{j}", [128, 32, 64], F32)) for j in range(2)]
            rg = [rctx.enter_context(nc.sbuf_tensor(f"rg{j}", [128, 32, 64], F32)) for j in range(2)]
            oh = [rctx.enter_context(nc.sbuf_tensor(f"oh{j}", [128, 32, 32], F32)) for j in range(2)]
            mm = [rctx.enter_context(nc.sbuf_tensor(f"mm{j}", [128, 32, 32], F32)) for j in range(2)]
            svr = [rctx.enter_context(nc.sbuf_tensor(f"svr{j}", [128, 32, 2], F32)) for j in range(2)]
            rvr = [rctx.enter_context(nc.sbuf_tensor(f"rvr{j}", [128, 32, 2], F32)) for j in range(2)]
            ld = rctx.enter_context(nc.semaphore("g_ld"))
            gs = rctx.enter_context(nc.semaphore("g_gs"))
            vs = rctx.enter_context(nc.semaphore("g_vs"))
            so = rctx.enter_context(nc.semaphore("g_so"))
            nc.gpsimd.load_library(_mlp_lib)
            dvec = [0]

            def dve_wait():
                if dvec[0]:
                    nc.vector.wait_ge(vs, dvec[0])

            def dve_done(inst):
                inst.then_inc(vs, 1)
                dvec[0] += 1

            # wait for table write, idx replicates and prep stores
            nc.gpsimd.wait_ge(tst, 144)
            nc.gpsimd.dma_start(i32t.ap()[:, :], io32f_d[:, :]).then_inc(ld, 16)
            nc.gpsimd.wait_ge(ld, 16)
            ldc = 16
            for g in range(cfg.N_GT):
                j = g % 2
                c0 = 32 * g
                fc = min(32, cfg.C_TOT - c0)
                ni = fc * 128
                if g >= 2:
                    nc.gpsimd.wait_ge(so, 32 * (g - 1))
                nc.gpsimd.dma_start(slot[j].ap()[:, :fc],
                                    slo_f_d[:, c0:c0 + fc]).then_inc(ld, 16)
                nc.gpsimd.dma_start(rlot[j].ap()[:, :fc],
                                    rlo_f_d[:, c0:c0 + fc]).then_inc(ld, 16)
                nc.gpsimd.dma_start(sbw[j].ap()[:, :fc * 8],
                                    sblkR[:, c0 * 8:(c0 + fc) * 8]
                                    ).then_inc(ld, 16)
                nc.gpsimd.dma_start(rbw[j].ap()[:, :fc * 8],
                                    rblkR[:, c0 * 8:(c0 + fc) * 8]
                                    ).then_inc(ld, 16)
                ldc += 64
                nc.gpsimd.wait_ge(ld, ldc)
                nc.gpsimd.dma_gather(
                    sg[j].ap()[:, :fc, :], table_v[:, :], sbw[j].ap()[:, :fc * 8],
                    ni, ni, 64, single_packet=False).then_inc(gs, 16)
                nc.gpsimd.dma_gather(
                    rg[j].ap()[:, :fc, :], table_v[:, :], rbw[j].ap()[:, :fc * 8],
                    ni, ni, 64, single_packet=False).then_inc(gs, 16)
                nc.vector.wait_ge(gs, 32 * (g + 1))
                nc.vector.wait_ge(ld, ldc)
                # sender select
                dve_wait()
                _i = nc.vector.tensor_tensor(
                    out=oh[j].ap()[:, :fc, :],
                    in0=slot[j].ap()[:, :fc].unsqueeze(2).to_broadcast([128, fc, 32]),
                    in1=i32t.ap()[:, :].unsqueeze(1).to_broadcast([128, fc, 32]),
                    op=TT.is_equal)
                dve_done(_i)
                dve_wait()
                _i = nc.vector.tensor_tensor(
                    out=mm[j].ap()[:, :fc, :], in0=oh[j].ap()[:, :fc, :],
                    in1=sg[j].ap()[:, :fc, 0::2], op=TT.mult)
                dve_done(_i)
                dve_wait()
                _i = nc.vector.reduce_sum(svr[j].ap()[:, :fc, 0:1],
                                          mm[j].ap()[:, :fc, :],
                                          axis=mybir.AxisListType.X)
                dve_done(_i)
                dve_wait()
                _i = nc.vector.tensor_tensor(
                    out=mm[j].ap()[:, :fc, :], in0=oh[j].ap()[:, :fc, :],
                    in1=sg[j].ap()[:, :fc, 1::2], op=TT.mult)
                dve_done(_i)
                dve_wait()
                _i = nc.vector.reduce_sum(svr[j].ap()[:, :fc, 1:2],
                                          mm[j].ap()[:, :fc, :],
                                          axis=mybir.AxisListType.X)
                dve_done(_i)
                # receiver select
                dve_wait()
                _i = nc.vector.tensor_tensor(
                    out=oh[j].ap()[:, :fc, :],
                    in0=rlot[j].ap()[:, :fc].unsqueeze(2).to_broadcast([128, fc, 32]),
                    in1=i32t.ap()[:, :].unsqueeze(1).to_broadcast([128, fc, 32]),
                    op=TT.is_equal)
                dve_done(_i)
                dve_wait()
                _i = nc.vector.tensor_tensor(
                    out=mm[j].ap()[:, :fc, :], in0=oh[j].ap()[:, :fc, :],
                    in1=rg[j].ap()[:, :fc, 0::2], op=TT.mult)
                dve_done(_i)
                dve_wait()
                _i = nc.vector.reduce_sum(rvr[j].ap()[:, :fc, 0:1],
                                          mm[j].ap()[:, :fc, :],
                                          axis=mybir.AxisListType.X)
                dve_done(_i)
                dve_wait()
                _i = nc.vector.tensor_tensor(
                    out=mm[j].ap()[:, :fc, :], in0=oh[j].ap()[:, :fc, :],
                    in1=rg[j].ap()[:, :fc, 1::2], op=TT.mult)
                dve_done(_i)
                dve_wait()
                _i = nc.vector.reduce_sum(
                    rvr[j].ap()[:, :fc, 1:2], mm[j].ap()[:, :fc, :],
                    axis=mybir.AxisListType.X)
                dve_done(_i)
                nc.gpsimd.wait_ge(vs, dvec[0])
                nc.gpsimd.dma_start(sv_all[:, c0:c0 + fc, :],
                                    svr[j].ap()[:, :fc, :]).then_inc(so, 16)
                nc.gpsimd.dma_start(rv_all[:, c0:c0 + fc, :],
                                    rvr[j].ap()[:, :fc, :]).then_inc(so, 16)
            nc.gpsimd.wait_ge(so, 32 * cfg.N_GT)
    nc.all_engine_barrier()

    # ---------------- edge phase (Tile) ----------------
    with tile.TileContext(nc) as tc, ExitStack() as ctx:
        consts = ctx.enter_context(tc.tile_pool(name="econsts", bufs=1))
        inp = ctx.enter_context(tc.tile_pool(name="einp", bufs=2))
        gat = ctx.enter_context(tc.tile_pool(name="egat", bufs=2))
        tmp = ctx.enter_context(tc.tile_pool(name="etmp", bufs=1))
        ohp = ctx.enter_context(tc.tile_pool(name="eoh", bufs=1))
        psum = ctx.enter_context(tc.tile_pool(name="epsum", bufs=1, space="PSUM"))
        psum2 = ctx.enter_context(tc.tile_pool(name="epsum2", bufs=1, space="PSUM"))

        ir = consts.tile([128, 128], I8)
        nc.sync.dma_start(ir[:], ir8_d[:])
        iq = consts.tile([128, cfg.QBINS], I8)
        nc.sync.dma_start(iq[:], iq8_d[:])
        idn = consts.tile([128, 128], F32)
        nc.sync.dma_start(idn[:], identf_d[:])
        eb = consts.tile([128, 4], F32)
        nc.sync.dma_start(eb[:], ebias[:])

        bins = psum.tile([128, cfg.QBINS], F32)

        n_mm = 0
        total_mm = cfg.C_TOT

        for t in range(n_tiles):
            c0 = t * F
            f = min(F, cfg.C_TOT - c0)
            lt16 = inp.tile([128, F], F16, name="lt16", tag="lt16")
            nc.sync.dma_start(lt16[:, :f], lens16[:, c0:c0 + f])
            mf = inp.tile([128, F], I8, name="mf", tag="mf")
            nc.sync.dma_start(mf[:, :f], m8d[:, c0:c0 + f])
            qf = inp.tile([128, F], I8, name="qf", tag="qf")
            nc.sync.dma_start(qf[:, :f], q8d[:, c0:c0 + f])

            sv = gat.tile([128, F, 2], F32, name="sv", tag="sv")
            nc.sync.dma_start(sv[:, :f, :], sv_all[:, c0:c0 + f, :])
            rv = gat.tile([128, F, 2], F32, name="rv", tag="rv")
            nc.sync.dma_start(rv[:, :f, :], rv_all[:, c0:c0 + f, :])

            als = sv[:, :f, 0]
            cs = sv[:, :f, 1]
            alr = rv[:, :f, 0]
            cr = rv[:, :f, 1]

            def T(tag):
                return tmp.tile([128, F], F32, name=tag, tag=tag)[:, :f]

            lt = T("ltf")
            nc.vector.tensor_copy(out=lt, in_=lt16[:, :f])

            a2 = T("a2"); nc.vector.tensor_add(out=a2, in0=als, in1=alr)
            u = T("u"); nc.vector.tensor_mul(out=u, in0=alr, in1=cs)
            tv = T("tv"); nc.vector.tensor_mul(out=tv, in0=als, in1=cr)
            ut = T("ut"); nc.vector.tensor_mul(out=ut, in0=u, in1=tv)
            du = T("du"); nc.vector.tensor_mul(out=du, in0=alr, in1=u)
            dt = T("dt"); nc.vector.tensor_mul(out=dt, in0=als, in1=tv)
            den = T("den"); nc.vector.tensor_add(out=den, in0=du, in1=dt)
            rden = T("rden"); nc.vector.reciprocal(out=rden, in_=den)
            c6p = T("c6p"); nc.vector.tensor_mul(out=c6p, in0=ut, in1=rden)

            la = T("la"); nc.scalar.activation(out=la, in_=a2, func=AF.Ln)
            q1 = T("q1"); nc.scalar.activation(out=q1, in_=la, func=AF.Exp,
                                               scale=1.0 / 7.0, bias=eb[:, 0:1])
            p6 = T("p6"); nc.scalar.activation(out=p6, in_=la, func=AF.Exp,
                                               scale=6.0 / 7.0, bias=eb[:, 1:2])
            p8 = T("p8"); nc.scalar.activation(out=p8, in_=la, func=AF.Exp,
                                               scale=8.0 / 7.0, bias=eb[:, 2:3])
            p10 = T("p10"); nc.scalar.activation(out=p10, in_=la, func=AF.Exp,
                                                 scale=10.0 / 7.0, bias=eb[:, 3:4])
            # s = b3 v^3 + b2 v^2 + b1 v + b0  (Horner)
            hh = T("hh"); nc.scalar.activation(out=hh, in_=q1, func=AF.Copy,
                                               scale=_GB3, bias=_GB2)
            h3 = T("h3"); nc.vector.tensor_mul(out=h3, in0=hh, in1=q1)
            nc.vector.tensor_scalar_add(out=h3, in0=h3, scalar1=_GB1)
            sres = T("sres"); nc.vector.tensor_mul(out=sres, in0=h3, in1=q1)
            nc.vector.tensor_scalar_add(out=sres, in0=sres, scalar1=_GB0)
            s2 = T("s2"); nc.vector.tensor_mul(out=s2, in0=sres, in1=sres)
            s4 = T("s4"); nc.vector.tensor_mul(out=s4, in0=s2, in1=s2)
            nc.vector.tensor_scalar_mul(out=s2, in0=s2, scalar1=10.0 * BOHR ** 2)
            nc.vector.tensor_scalar_mul(out=s4, in0=s4, scalar1=122.5 * BOHR ** 4)

            l2 = T("l2"); nc.vector.tensor_mul(out=l2, in0=lt, in1=lt)
            l4 = T("l4"); nc.vector.tensor_mul(out=l4, in0=l2, in1=l2)
            l6 = T("l6"); nc.vector.tensor_mul(out=l6, in0=l4, in1=l2)
            l8 = T("l8"); nc.vector.tensor_mul(out=l8, in0=l4, in1=l4)
            l10 = T("l10"); nc.vector.tensor_mul(out=l10, in0=l6, in1=l4)
            nc.vector.tensor_add(out=l6, in0=l6, in1=p6)
            nc.vector.tensor_add(out=l8, in0=l8, in1=p8)
            nc.vector.tensor_add(out=l10, in0=l10, in1=p10)
            r6 = T("r6"); nc.vector.reciprocal(out=r6, in_=l6)
            r8 = T("r8"); nc.vector.reciprocal(out=r8, in_=l8)
            r10 = T("r10"); nc.vector.reciprocal(out=r10, in_=l10)
            m8_ = T("m8_"); nc.vector.tensor_mul(out=m8_, in0=s2, in1=r8)
            m10 = T("m10"); nc.vector.tensor_mul(out=m10, in0=s4, in1=r10)
            nc.vector.tensor_add(out=r6, in0=r6, in1=m8_)
            nc.vector.tensor_add(out=r6, in0=r6, in1=m10)
            epre = T("epre"); nc.vector.tensor_mul(out=epre, in0=c6p, in1=r6)
            nc.vector.tensor_scalar_mul(out=epre, in0=epre,
                                        scalar1=-2.0 * _C6F)

            # switching function
            cx = T("cx"); nc.scalar.activation(out=cx, in_=lt, func=AF.Copy,
                                               scale=0.5, bias=-4.0)
            x1 = T("x1"); nc.scalar.activation(out=x1, in_=cx, func=AF.Copy,
                                               scale=-1.0, bias=1.0)
            nc.vector.tensor_scalar_max(out=x1, in0=x1, scalar1=1e-12)
            x2 = T("x2"); nc.vector.tensor_scalar_max(out=x2, in0=cx, scalar1=1e-12)
            n1 = T("n1"); nc.vector.reciprocal(out=n1, in_=x1)
            n2 = T("n2"); nc.vector.reciprocal(out=n2, in_=x2)
            nc.vector.tensor_scalar_min(out=n1, in0=n1, scalar1=87.0)
            nc.vector.tensor_scalar_min(out=n2, in0=n2, scalar1=87.0)
            e1 = T("e1"); nc.scalar.activation(out=e1, in_=n1, func=AF.Exp, scale=-1.0)
            e2 = T("e2"); nc.scalar.activation(out=e2, in_=n2, func=AF.Exp, scale=-1.0)
            ws = T("ws"); nc.vector.tensor_add(out=ws, in0=e1, in1=e2)
            nc.vector.tensor_scalar_add(out=ws, in0=ws, scalar1=1e-12)
            rw = T("rw"); nc.vector.reciprocal(out=rw, in_=ws)
            wv = T("wv"); nc.vector.tensor_mul(out=wv, in0=e1, in1=rw)
            v = T("v"); nc.vector.tensor_mul(out=v, in0=epre, in1=wv)

            # scatter: one-hot matmuls, batches of 32 columns
            BW = 32
            for b0 in range(0, f, BW):
                bw = min(BW, f - b0)
                ohr = ohp.tile([128, BW, 128], F32, name="ohr", tag="ohr")
                nc.vector.tensor_tensor(
                    out=ohr[:, :bw, :],
                    in0=mf[:, b0:b0 + bw].unsqueeze(2).to_broadcast([128, bw, 128]),
                    in1=ir[:].unsqueeze(1).to_broadcast([128, bw, 128]),
                    op=mybir.AluOpType.is_equal)
                ohq = ohp.tile([128, BW, cfg.QBINS], F32, name="ohq", tag="ohq")
                nc.vector.tensor_tensor(
                    out=ohq[:, :bw, :],
                    in0=qf[:, b0:b0 + bw].unsqueeze(2).to_broadcast(
                        [128, bw, cfg.QBINS]),
                    in1=iq[:].unsqueeze(1).to_broadcast([128, bw, cfg.QBINS]),
                    op=mybir.AluOpType.is_equal)
                nc.vector.tensor_tensor(
                    out=ohq[:, :bw, :],
                    in0=ohq[:, :bw, :],
                    in1=v[:, b0:b0 + bw].unsqueeze(2).to_broadcast(
                        [128, bw, cfg.QBINS]),
                    op=mybir.AluOpType.mult)
                for j in range(bw):
                    nc.tensor.matmul(
                        bins[:, :], lhsT=ohr[:, j, :], rhs=ohq[:, j, :],
                        start=(n_mm == 0), stop=(n_mm == total_mm - 1))
                    n_mm += 1

        # transpose bins [128, QBINS] -> [QBINS, 128] and write out
        bsb = consts.tile([128, cfg.QBINS], F32)
        nc.vector.tensor_copy(out=bsb[:], in_=bins[:])
        btp = psum2.tile([128, 128], F32)
        nc.tensor.transpose(out=btp[:cfg.QBINS, :], in_=bsb[:], identity=idn[:])
        bts = consts.tile([cfg.QBINS, 128], F32)
        nc.vector.tensor_copy(out=bts[:], in_=btp[:cfg.QBINS, :])
        nc.sync.dma_start(out[:, :], bts[:])

    nc.compile()
    return nc


_NC_CACHE = {}
_EXEC_CACHE = {}


def _get_nc(cfg):
    key = (cfg.N, cfg.C_TOT)
    if key not in _NC_CACHE:
        _NC_CACHE[key] = build_nc(cfg)
    return _NC_CACHE[key]


class _SpmdExec:
    """Cached shard_map execution of a Bass NEFF on n cores via PJRT.

    Mirrors concourse.bass2jax.run_bass_via_pjrt but keeps the jitted
    callable (and its lowering) across calls, so repeated runs only pay
    input transfer + device execution.
    """

    def __init__(self, nc, n_cores):
        from concourse import bass2jax
        bass2jax.install_neuronx_cc_hook()
        assert nc.dbg_addr is None or not nc.dbg_callbacks
        self.nc = nc
        self.n = n_cores
        partition_name = (nc.partition_id_tensor.name
                          if nc.partition_id_tensor else None)
        in_names, out_names, out_avals = [], [], []
        for alloc in nc.m.functions[0].allocations:
            if not isinstance(alloc, mybir.MemoryLocationSet):
                continue
            name = alloc.memorylocations[0].name
            if alloc.kind == "ExternalInput":
                if name != partition_name:
                    in_names.append(name)
            elif alloc.kind == "ExternalOutput":
                shape = tuple(alloc.tensor_shape)
                dtype = mybir.dt.np(alloc.dtype)
                out_names.append(name)
                out_avals.append(jax.core.ShapedArray(shape, dtype))
        self.dbg_name = nc.dbg_addr.name if nc.dbg_addr is not None else None
        self.in_names = list(in_names)
        if self.dbg_name is not None and self.dbg_name in self.in_names:
            pass  # already an ExternalInput; supplied as zeros in run()
        self.out_names = out_names
        self.out_avals = out_avals
        n_params = len(self.in_names)
        all_names = self.in_names + out_names
        if partition_name is not None:
            all_names.append(partition_name)
        donate = tuple(range(n_params, n_params + len(out_names)))
        bind_names = tuple(all_names)

        def _body(*args):
            operands = list(args)
            if partition_name is not None:
                operands.append(bass2jax.partition_id_tensor())
            outs = bass2jax._bass_exec_p.bind(
                *operands,
                out_avals=tuple(out_avals),
                in_names=bind_names,
                out_names=tuple(out_names),
                lowering_input_output_aliases=(),
                sim_require_finite=True,
                sim_require_nnan=True,
                nc=nc,
            )
            return tuple(outs)

        devices = jax.devices()[:n_cores]
        assert len(devices) == n_cores
        self.mesh = Mesh(np.asarray(devices), ("core",))
        in_specs = (PartitionSpec("core"),) * (n_params + len(out_names))
        out_specs = (PartitionSpec("core"),) * len(out_names)
        self.fn = jax.jit(
            shard_map(_body, mesh=self.mesh, in_specs=in_specs,
                      out_specs=out_specs, check_rep=False),
            donate_argnums=donate, keep_unused=True)

    def run(self, in_maps):
        n = self.n
        if self.dbg_name is not None:
            z = np.zeros((1, 2), np.uint32)
            in_maps = [{**m, self.dbg_name: z} for m in in_maps]
        concat_in = [
            np.concatenate([np.asarray(m[name]) for m in in_maps], axis=0)
            for name in self.in_names
        ]
        zeros = [np.zeros((n * a.shape[0], *a.shape[1:]), a.dtype)
                 for a in self.out_avals]
        outs = self.fn(*concat_in, *zeros)
        return [
            {name: np.asarray(outs[i]).reshape(n, *self.out_avals[i].shape)[c]
             for i, name in enumerate(self.out_names)}
            for c in range(n)
        ]


def _get_exec(cfg):
    key = (cfg.N, cfg.C_TOT)
    if key not in _EXEC_CACHE:
        _EXEC_CACHE[key] = _SpmdExec(_get_nc(cfg), NCORES)
    return _EXEC_CACHE[key]


def shard_inputs(cfg, hirshfeld_ratios, atomic_numbers, senders_lr,
                 receivers_lr, lengths_lr):
    N, W, EPAD, C_TOT = cfg.N, cfg.W, cfg.EPAD, cfg.C_TOT
    h = np.asarray(hirshfeld_ratios, np.float32)
    z = np.asarray(atomic_numbers, np.int32)
    s = np.asarray(senders_lr, np.int32)
    r = np.asarray(receivers_lr, np.int32)
    ln = np.asarray(lengths_lr, np.float32)

    hp = np.ones(cfg.NPAD, np.float32)
    hp[:N] = h
    zp = np.ones(cfg.NPAD, np.int32)
    zp[:N] = z
    h16 = hp.reshape(128, cfg.NODE_F).astype(np.float16)
    z8 = (zp - 1).astype(np.int8).reshape(128, cfg.NODE_F).T.copy().reshape(-1)
    ac_tab = np.zeros((128, 2), np.float32)
    ac_tab[:len(ALPHAS), 0] = ALPHAS
    ac_tab[:len(C6_COEF), 1] = C6_COEF
    iota_col = np.arange(128, dtype=np.float32).reshape(128, 1)
    ebias = np.tile(np.array([[_B1, _B6, _B8, _B10]], np.float32), (128, 1))

    core_of = r // W
    order = np.argsort(core_of, kind="stable")
    s_o, r_o, l_o, c_o = s[order], r[order], ln[order], core_of[order]
    bounds = np.searchsorted(c_o, np.arange(NCORES + 1))

    def wrap_blk(arr):
        blk2 = (arr >> 5).astype(np.int16).reshape(128, C_TOT)
        parts = []
        for g in range(cfg.N_GT):
            c0 = 32 * g
            fc = min(32, C_TOT - c0)
            unw = blk2[:, c0:c0 + fc].T.reshape(-1)       # i = c*128+p
            parts.append(unw.reshape(fc * 8, 16).T)       # [16, fc*8]
        return np.ascontiguousarray(np.concatenate(parts, axis=1))

    in_maps = []
    for c in range(NCORES):
        lo, hi = bounds[c], bounds[c + 1]
        cnt = hi - lo
        assert cnt <= EPAD, f"core {c} edge count {cnt} > EPAD {EPAD}"
        base = c * W
        sp = np.zeros(EPAD, np.int32)
        rp = np.full(EPAD, base, np.int32)
        lp = np.full(EPAD, 100.0, np.float32)
        sp[:cnt] = s_o[lo:hi]
        rp[:cnt] = r_o[lo:hi]
        lp[:cnt] = l_o[lo:hi]
        rloc = rp - base
        in_maps.append({
            "h16": h16, "z8": z8, "ac_tab": ac_tab, "iota_col": iota_col,
            "ebias": ebias,
            "b0s": np.full((128, 1), base & 31, np.int32),
            "sblk16": wrap_blk(sp), "rblk16": wrap_blk(rp),
            "slo8": (sp & 31).astype(np.int8).reshape(128, C_TOT),
            "m8": (rloc & 127).astype(np.int8).reshape(128, C_TOT),
            "q8": (rloc >> 7).astype(np.int8).reshape(128, C_TOT),
            "lens16": lp.astype(np.float16).reshape(128, C_TOT),
        })
    return in_maps


def unshard(cfg, results):
    outp = np.zeros(cfg.N, np.float32)
    for c in range(NCORES):
        o = results[c]["out"].reshape(-1)[:cfg.W]
        outp[c * cfg.W:(c + 1) * cfg.W] = o
    return outp.reshape(-1, 1)


def run_all(cfg, in_maps):
    ex = _get_exec(cfg)
    return ex.run(in_maps)


def kernel(hirshfeld_ratios, atomic_numbers, senders_lr, receivers_lr,
           lengths_lr, num_nodes):
    cfg = FULL
    assert int(num_nodes) == cfg.N
    in_maps = shard_inputs(cfg, hirshfeld_ratios, atomic_numbers, senders_lr,
                           receivers_lr, lengths_lr)
    results = run_all(cfg, in_maps)
    return unshard(cfg, results)


# revision 19
# speedup vs baseline: 5.9273x; 1.0001x over previous
"""Trainium2 Bass kernel for nn_DispersionInteraction (vdW-QDO dispersion).

Strategy (8 NeuronCores, SPMD single NEFF):
  - Edges are sharded across cores by RECEIVER block (core c owns nodes
    [c*12500, (c+1)*12500)), so each core's local segment-sum covers only
    12544 bins and no cross-core reduction is needed (outputs concatenate).
  - Single NEFF per core, four phases:
      1. Node phase (Tile): builds the (alpha_n, C6_n) = (A[z]*h, C[z]*h^2)
         table on-device via one-hot matmul on the tensor engine, into a raw
         SBUF buffer.
      2. Prep phase (raw): writes the table to Internal DRAM, replicates the
         [16, E/16] wrapped gather indices to the 128-partition layout
         dma_gather needs, expands int8 sideband inputs (slo, rlo derived
         from m) to f32 selection keys, and generates iota/identity tensors
         on-device (gpsimd iota) so none of them are uploaded.
      3. Gather phase (raw): per-edge (alpha, C6) records for sender and
         receiver fetched with the GPSIMD dma_gather ucode op at 32-node
         block granularity (256B rows, int16 block ids), then the right 8B
         record selected on the vector engine with a one-hot over the low 5
         index bits.
      4. Edge phase (Tile): per-edge energies via DVE/ACT ops; segment-sum
         on the tensor engine via one-hot matmuls accumulating into a PSUM
         [128, 98] bin grid (bin = (r_local & 127, r_local >> 7)).
  - Host->device payload is minimized (~9B/edge vs 52B/edge naive): block
    ids as non-replicated int16, 5/7-bit sidebands as int8, lengths as fp16.
    The axon PJRT tunnel is ~40-70 MB/s, so upload bytes dominate wall time.
  - Execution uses a cached jit of the shard_map'd bass_exec call, so
    repeated runs don't re-trace or re-lower.
"""

import math
import sys

import numpy as np

sys.path.insert(0, "/opt/trn_rl_repo")

import jax
from jax.experimental.shard_map import shard_map
from jax.sharding import Mesh, PartitionSpec

import concourse.bass as bass
import concourse.tile as tile
from concourse import bacc, mybir
from contextlib import ExitStack

F32 = mybir.dt.float32
F16 = mybir.dt.float16
I32 = mybir.dt.int32
I16 = mybir.dt.int16
I8 = mybir.dt.int8

BOHR = 0.5291772105638411
FINE_STRUCTURE = 0.0072973525693
HARTREE = 27.211386245988
C_FACTOR = 0.5

ALPHAS = np.array([4.5, 1.38, 164.2, 38.0, 21.0, 12.0, 7.4, 5.4, 3.8, 2.67, 162.7, 71.0, 60.0, 37.0, 25.0, 19.6, 15.0, 11.1, 292.9, 160.0, 120.0, 98.0, 84.0, 78.0, 63.0, 56.0, 50.0, 48.0, 42.0, 40.0, 60.0, 41.0, 29.0, 25.0, 20.0, 16.8, 319.2, 199.0, 126.74, 119.97, 101.6, 88.42, 80.08, 65.89, 56.1, 23.68, 50.6, 39.7, 70.22, 55.95, 43.67, 37.65, 35.0, 27.3, 399.9, 275.0, 213.7, 204.7, 215.8, 208.4, 200.2, 192.1, 184.2, 158.3, 169.5, 164.64, 156.3, 150.2, 144.3, 138.9, 137.2, 99.52, 82.53, 71.04, 63.04, 55.06, 42.51, 39.68, 36.5, 33.9, 69.92, 61.8, 49.02, 45.01, 38.93, 33.54, 317.8, 246.2, 203.3, 217.0, 154.4, 127.8, 150.5, 132.2, 131.2, 143.6, 125.3, 121.5, 117.5, 113.4, 109.4, 105.4], dtype=np.float32)
C6_COEF = np.array([6.5, 1.46, 1387.0, 214.0, 99.5, 46.6, 24.2, 15.6, 9.52, 6.38, 1556.0, 627.0, 528.0, 305.0, 185.0, 134.0, 94.6, 64.3, 3897.0, 2221.0, 1383.0, 1044.0, 832.0, 602.0, 552.0, 482.0, 408.0, 373.0, 253.0, 284.0, 498.0, 354.0, 246.0, 210.0, 162.0, 129.6, 4691.0, 3170.0, 1968.58, 1677.91, 1263.61, 1028.73, 1390.87, 609.75, 469.0, 157.5, 339.0, 452.0, 707.05, 587.42, 459.32, 396.0, 385.0, 285.9, 6846.0, 5727.0, 3884.5, 3708.33, 3911.84, 3908.75, 3847.68, 3708.69, 3511.71, 2781.53, 3124.41, 2984.29, 2839.95, 2724.12, 2576.78, 2387.53, 2371.8, 1274.8, 1019.92, 847.93, 710.2, 596.67, 359.1, 347.1, 298.0, 392.0, 717.44, 697.0, 571.0, 530.92, 457.53, 390.63, 4224.44, 4851.32, 3604.41, 4047.54, 2876.77, 2375.89, 3102.12, 2820.47, 2794.0, 3150.95, 2756.0, 2702.57, 2626.59, 2548.62, 2468.69, 2386.8], dtype=np.float32)

NCORES = 8


class Cfg:
    def __init__(self, n_nodes, e_total, c_tot):
        self.N = n_nodes
        self.W = n_nodes // NCORES          # nodes owned per core
        self.NODE_F = math.ceil(n_nodes / 128 / 4) * 4   # free cols, mult of 4
        self.NPAD = 128 * self.NODE_F
        assert self.NPAD % 512 == 0
        self.NCHUNK = self.NPAD // 512
        self.QBINS = math.ceil(self.W / 128)
        self.BINS = 128 * self.QBINS
        self.C_TOT = c_tot                   # edge columns per core
        self.EPAD = 128 * c_tot
        self.F = min(512, c_tot)             # columns per edge tile
        self.N_GT = (c_tot + 31) // 32       # gather groups of 32 cols
        self.WC = c_tot * 8                  # wrapped idx cols


FULL = Cfg(100000, 6400000, 6320)

# folded constants
_PB = 2.0 * 2.54 * BOHR          # p * BOHR = _PB * alpha_ij^{1/7}
_C6F = C_FACTOR * HARTREE * BOHR ** 6
_B1 = math.log(FINE_STRUCTURE ** (-4.0 / 21.0)) - math.log(2.0) / 7.0
_B6 = 6.0 * math.log(_PB) - 6.0 * math.log(2.0) / 7.0
_B8 = 8.0 * math.log(_PB) - 8.0 * math.log(2.0) / 7.0
_B10 = 10.0 * math.log(_PB) - 10.0 * math.log(2.0) / 7.0
_GB0, _GB1, _GB2, _GB3 = -0.00433008, 0.24428889, 0.04125273, -0.00078893


def build_nc(cfg: Cfg):
    nc = bacc.Bacc("TRN2")
    F = cfg.F
    n_tiles = (cfg.C_TOT + F - 1) // F

    # ---- inputs ----
    h16 = nc.dram_tensor("h16", [128, cfg.NODE_F], F16, kind="ExternalInput")
    z8d = nc.dram_tensor("z8", [cfg.NPAD], I8, kind="ExternalInput")
    ac_tab = nc.dram_tensor("ac_tab", [128, 2], F32, kind="ExternalInput")
    iota_col = nc.dram_tensor("iota_col", [128, 1], F32, kind="ExternalInput")
    sblk16 = nc.dram_tensor("sblk16", [16, cfg.WC], I16, kind="ExternalInput")
    rblk16 = nc.dram_tensor("rblk16", [16, cfg.WC], I16, kind="ExternalInput")
    slo8d = nc.dram_tensor("slo8", [128, cfg.C_TOT], I8, kind="ExternalInput")
    m8d = nc.dram_tensor("m8", [128, cfg.C_TOT], I8, kind="ExternalInput")
    q8d = nc.dram_tensor("q8", [128, cfg.C_TOT], I8, kind="ExternalInput")
    lens16 = nc.dram_tensor("lens16", [128, cfg.C_TOT], F16, kind="ExternalInput")
    bs32d = nc.dram_tensor("bs32", [128, 1], I32, kind="ExternalInput")
    ebias = nc.dram_tensor("ebias", [128, 4], F32, kind="ExternalInput")
    out = nc.dram_tensor("out", [cfg.QBINS, 128], F32, kind="ExternalOutput")

    # ---- internals ----
    table = nc.dram_tensor("table", [cfg.NPAD, 2], F32, kind="Internal")
    sblkR = nc.dram_tensor("sblkR", [128, cfg.WC], I16, kind="Internal")
    rblkR = nc.dram_tensor("rblkR", [128, cfg.WC], I16, kind="Internal")
    slo_f_d = nc.dram_tensor("slo_f_d", [128, cfg.C_TOT], F32, kind="Internal")
    rlo_f_d = nc.dram_tensor("rlo_f_d", [128, cfg.C_TOT], F32, kind="Internal")
    io32f_d = nc.dram_tensor("io32f_d", [128, 32], F32, kind="Internal")
    ir8_d = nc.dram_tensor("ir8_d", [128, 128], I8, kind="Internal")
    iq8_d = nc.dram_tensor("iq8_d", [128, cfg.QBINS], I8, kind="Internal")
    identf_d = nc.dram_tensor("identf_d", [128, 128], F32, kind="Internal")
    sv_all = nc.dram_tensor("sv_all", [128, cfg.C_TOT, 2], F32, kind="Internal")
    rv_all = nc.dram_tensor("rv_all", [128, cfg.C_TOT, 2], F32, kind="Internal")

    TT = mybir.AluOpType
    AF = mybir.ActivationFunctionType

    with ExitStack() as octx:
        # node table staging buffer, lives across the node tc + prep phase
        acn = octx.enter_context(
            nc.sbuf_tensor("acn_raw", [128, cfg.NODE_F, 2], F32))

        # ---------------- node phase (Tile) ----------------
        with tile.TileContext(nc) as tc, ExitStack() as ctx:
            consts = ctx.enter_context(tc.tile_pool(name="nconsts", bufs=1))
            pool = ctx.enter_context(tc.tile_pool(name="npool", bufs=3))
            psum = ctx.enter_context(tc.tile_pool(name="npsum", bufs=3,
                                                  space="PSUM"))
            ic = consts.tile([128, 1], F32)
            nc.sync.dma_start(ic[:], iota_col[:])
            act = consts.tile([128, 2], F32)
            nc.sync.dma_start(act[:], ac_tab[:])
            hn = consts.tile([128, cfg.NODE_F], F16)
            nc.sync.dma_start(hn[:], h16[:])
            hf = consts.tile([128, cfg.NODE_F], F32)
            nc.vector.tensor_copy(out=hf[:], in_=hn[:])

            for c in range(cfg.NCHUNK):
                zb8 = pool.tile([128, 512], I8, name="zb8", tag="zb8")
                nc.sync.dma_start(
                    zb8[:], z8d[None, 512 * c:512 * (c + 1)]
                    .to_broadcast([128, 512]))
                zbf = pool.tile([128, 512], F32, name="zbf", tag="zbf")
                nc.vector.tensor_copy(out=zbf[:], in_=zb8[:])
                oh = pool.tile([128, 512], F32, name="oh", tag="oh")
                nc.vector.tensor_tensor(
                    out=oh[:], in0=zbf[:], in1=ic[:].to_broadcast([128, 512]),
                    op=TT.is_equal)
                ps = psum.tile([128, 4, 2], F32, name="ps", tag="ps")
                for j in range(4):
                    nc.tensor.matmul(ps[:, j, :],
                                     lhsT=oh[:, 128 * j:128 * (j + 1)],
                                     rhs=act[:], start=True, stop=True)
                nc.vector.tensor_copy(
                    out=acn.ap()[:, 4 * c:4 * c + 4, :], in_=ps[:, :, :])
            # alpha = A*h ; C6 = C*h^2  (acn is raw SBUF; DVE is in-order)
            h2 = consts.tile([128, cfg.NODE_F], F32)
            nc.vector.tensor_mul(out=h2[:], in0=hf[:], in1=hf[:])
            nc.vector.tensor_mul(out=acn.ap()[:, :, 0],
                                 in0=acn.ap()[:, :, 0], in1=hf[:])
            nc.vector.tensor_mul(out=acn.ap()[:, :, 1],
                                 in0=acn.ap()[:, :, 1], in1=h2[:])
        nc.all_engine_barrier()

        # ---------------- prep + gather phase (raw) ----------------
        from concourse.library_config import mlp as _mlp_lib
        table_v = table.rearrange("(b w) c -> b (w c)", w=32)
        with ExitStack() as rctx:
            # prep tensors
            slo8s = rctx.enter_context(
                nc.sbuf_tensor("slo8s", [128, cfg.C_TOT], I8))
            m8s = rctx.enter_context(
                nc.sbuf_tensor("m8s", [128, cfg.C_TOT], I8))
            q8s = rctx.enter_context(
                nc.sbuf_tensor("q8s", [128, cfg.C_TOT], I8))
            bs32s = rctx.enter_context(nc.sbuf_tensor("bs32s", [128, 1], I32))
            m32 = rctx.enter_context(
                nc.sbuf_tensor("m32", [128, cfg.C_TOT], I32))
            r32 = rctx.enter_context(
                nc.sbuf_tensor("r32", [128, cfg.C_TOT], I32))
            slo_f = rctx.enter_context(
                nc.sbuf_tensor("slo_f", [128, cfg.C_TOT], F32))
            rlo_f = rctx.enter_context(
                nc.sbuf_tensor("rlo_f", [128, cfg.C_TOT], F32))
            io32 = rctx.enter_context(nc.sbuf_tensor("io32", [128, 32], I32))
            io32f = rctx.enter_context(nc.sbuf_tensor("io32f", [128, 32], F32))
            ior32 = rctx.enter_context(nc.sbuf_tensor("ior32", [128, 128], I32))
            iorP = rctx.enter_context(nc.sbuf_tensor("iorP", [128, 128], I32))
            ioq32 = rctx.enter_context(
                nc.sbuf_tensor("ioq32", [128, cfg.QBINS], I32))
            ir8 = rctx.enter_context(nc.sbuf_tensor("ir8", [128, 128], I8))
            iq8 = rctx.enter_context(
                nc.sbuf_tensor("iq8", [128, cfg.QBINS], I8))
            identf = rctx.enter_context(
                nc.sbuf_tensor("identf", [128, 128], F32))

            tld = rctx.enter_context(nc.semaphore("tld"))
            tio = rctx.enter_context(nc.semaphore("tio"))
            tdv = rctx.enter_context(nc.semaphore("tdv"))
            tst = rctx.enter_context(nc.semaphore("tst"))

            # prep loads
            nc.sync.dma_start(slo8s.ap()[:, :], slo8d[:, :]).then_inc(tld, 16)
            nc.sync.dma_start(m8s.ap()[:, :], m8d[:, :]).then_inc(tld, 16)
            nc.sync.dma_start(q8s.ap()[:, :], q8d[:, :]).then_inc(tld, 16)
            nc.sync.dma_start(bs32s.ap()[:, :], bs32d[:, :]).then_inc(tld, 16)

            # stores with no prep dependency: table + idx replicates
            nc.sync.dma_start(
                table.rearrange("(p f) c -> p f c", p=128),
                acn.ap()[:, :, :]).then_inc(tst, 16)
            nc.sync.dma_start(
                sblkR.rearrange("(a b) x -> a b x", a=8),
                sblk16[None, :, :].to_broadcast([8, 16, cfg.WC])
            ).then_inc(tst, 16)
            nc.sync.dma_start(
                rblkR.rearrange("(a b) x -> a b x", a=8),
                rblk16[None, :, :].to_broadcast([8, 16, cfg.WC])
            ).then_inc(tst, 16)

            # iotas on gpsimd
            nc.gpsimd.iota(io32.ap()[:, :], pattern=[[1, 32]], base=0,
                           channel_multiplier=0).then_inc(tio, 1)
            nc.gpsimd.iota(ior32.ap()[:, :], pattern=[[1, 128]], base=0,
                           channel_multiplier=0).then_inc(tio, 1)
            nc.gpsimd.iota(iorP.ap()[:, :], pattern=[[0, 128]], base=0,
                           channel_multiplier=1).then_inc(tio, 1)
            nc.gpsimd.iota(ioq32.ap()[:, :], pattern=[[1, cfg.QBINS]], base=0,
                           channel_multiplier=0).then_inc(tio, 1)

            # DVE prep chain (in order): r = (q<<7) + m + base
            nc.vector.wait_ge(tld, 64)
            nc.vector.wait_ge(tio, 4)
            nc.vector.tensor_copy(out=slo_f.ap()[:, :], in_=slo8s.ap()[:, :])
            nc.vector.tensor_copy(out=m32.ap()[:, :], in_=m8s.ap()[:, :])
            nc.vector.tensor_copy(out=r32.ap()[:, :], in_=q8s.ap()[:, :])
            nc.vector.tensor_scalar(out=r32.ap()[:, :], in0=r32.ap()[:, :],
                                    scalar1=7, scalar2=None,
                                    op0=TT.logical_shift_left)
            nc.vector.tensor_tensor(out=r32.ap()[:, :], in0=r32.ap()[:, :],
                                    in1=m32.ap()[:, :], op=TT.add)
            nc.vector.tensor_tensor(
                out=r32.ap()[:, :], in0=r32.ap()[:, :],
                in1=bs32s.ap()[:, 0:1].to_broadcast([128, cfg.C_TOT]),
                op=TT.add)
            nc.vector.tensor_scalar(out=r32.ap()[:, :], in0=r32.ap()[:, :],
                                    scalar1=31, scalar2=None,
                                    op0=TT.bitwise_and)
            nc.vector.tensor_copy(out=rlo_f.ap()[:, :], in_=r32.ap()[:, :])
            nc.vector.tensor_copy(out=io32f.ap()[:, :], in_=io32.ap()[:, :])
            nc.vector.tensor_copy(out=ir8.ap()[:, :], in_=ior32.ap()[:, :])
            nc.vector.tensor_copy(out=iq8.ap()[:, :], in_=ioq32.ap()[:, :])
            nc.vector.tensor_tensor(out=identf.ap()[:, :], in0=iorP.ap()[:, :],
                                    in1=ior32.ap()[:, :],
                                    op=TT.is_equal).then_inc(tdv, 1)

            # prep stores
            nc.sync.wait_ge(tdv, 1)
            nc.sync.dma_start(slo_f_d[:, :], slo_f.ap()[:, :]).then_inc(tst, 16)
            nc.sync.dma_start(rlo_f_d[:, :], rlo_f.ap()[:, :]).then_inc(tst, 16)
            nc.sync.dma_start(io32f_d[:, :], io32f.ap()[:, :]).then_inc(tst, 16)
            nc.sync.dma_start(ir8_d[:, :], ir8.ap()[:, :]).then_inc(tst, 16)
            nc.sync.dma_start(iq8_d[:, :], iq8.ap()[:, :]).then_inc(tst, 16)
            nc.sync.dma_start(identf_d[:, :], identf.ap()[:, :]
                              ).then_inc(tst, 16)

            # ------------- gather section (dma_gather block-32 + select) ----
            sbw = [rctx.enter_context(nc.sbuf_tensor(f"sbw{j}", [128, 32 * 8], I16)) for j in range(2)]
            rbw = [rctx.enter_context(nc.sbuf_tensor(f"rbw{j}", [128, 32 * 8], I16)) for j in range(2)]
            i32t = rctx.enter_context(nc.sbuf_tensor("i32t", [128, 32], F32))
            slot = [rctx.enter_context(nc.sbuf_tensor(f"slot{j}", [128, 32], F32)) for j in range(2)]
            rlot = [rctx.enter_context(nc.sbuf_tensor(f"rlot{j}", [128, 32], F32)) for j in range(2)]
            sg = [rctx.enter_context(nc.sbuf_tensor(f"sg{j}", [128, 32, 64], F32)) for j in range(2)]
            rg = [rctx.enter_context(nc.sbuf_tensor(f"rg{j}", [128, 32, 64], F32)) for j in range(2)]
            oh = [rctx.enter_context(nc.sbuf_tensor(f"oh{j}", [128, 32, 32], F32)) for j in range(2)]
            mm = [rctx.enter_context(nc.sbuf_tensor(f"mm{j}", [128, 32, 32], F32)) for j in range(2)]
            svr = [rctx.enter_context(nc.sbuf_tensor(f"svr{j}", [128, 32, 2], F32)) for j in range(2)]
            rvr = [rctx.enter_context(nc.sbuf_tensor(f"rvr{j}", [128, 32, 2], F32)) for j in range(2)]
            ld = rctx.enter_context(nc.semaphore("g_ld"))
            gs = rctx.enter_context(nc.semaphore("g_gs"))
            vs = rctx.enter_context(nc.semaphore("g_vs"))
            so = rctx.enter_context(nc.semaphore("g_so"))
            nc.gpsimd.load_library(_mlp_lib)
            dvec = [0]

            def dve_wait():
                if dvec[0]:
                    nc.vector.wait_ge(vs, dvec[0])

            def dve_done(inst):
                inst.then_inc(vs, 1)
                dvec[0] += 1

            # wait for table write, idx replicates and prep stores
            nc.gpsimd.wait_ge(tst, 144)
            nc.gpsimd.dma_start(i32t.ap()[:, :], io32f_d[:, :]).then_inc(ld, 16)
            nc.gpsimd.wait_ge(ld, 16)
            ldc = 16
            for g in range(cfg.N_GT):
                j = g % 2
                c0 = 32 * g
                fc = min(32, cfg.C_TOT - c0)
                ni = fc * 128
                if g >= 2:
                    nc.gpsimd.wait_ge(so, 32 * (g - 1))
                nc.gpsimd.dma_start(slot[j].ap()[:, :fc],
                                    slo_f_d[:, c0:c0 + fc]).then_inc(ld, 16)
                nc.gpsimd.dma_start(rlot[j].ap()[:, :fc],
                                    rlo_f_d[:, c0:c0 + fc]).then_inc(ld, 16)
                nc.gpsimd.dma_start(sbw[j].ap()[:, :fc * 8],
                                    sblkR[:, c0 * 8:(c0 + fc) * 8]
                                    ).then_inc(ld, 16)
                nc.gpsimd.dma_start(rbw[j].ap()[:, :fc * 8],
                                    rblkR[:, c0 * 8:(c0 + fc) * 8]
                                    ).then_inc(ld, 16)
                ldc += 64
                nc.gpsimd.wait_ge(ld, ldc)
                nc.gpsimd.dma_gather(
                    sg[j].ap()[:, :fc, :], table_v[:, :], sbw[j].ap()[:, :fc * 8],
                    ni, ni, 64, single_packet=False).then_inc(gs, 16)
                nc.gpsimd.dma_gather(
                    rg[j].ap()[:, :fc, :], table_v[:, :], rbw[j].ap()[:, :fc * 8],
                    ni, ni, 64, single_packet=False).then_inc(gs, 16)
                nc.vector.wait_ge(gs, 32 * (g + 1))
                nc.vector.wait_ge(ld, ldc)
                # sender select
                dve_wait()
                _i = nc.vector.tensor_tensor(
                    out=oh[j].ap()[:, :fc, :],
                    in0=slot[j].ap()[:, :fc].unsqueeze(2).to_broadcast([128, fc, 32]),
                    in1=i32t.ap()[:, :].unsqueeze(1).to_broadcast([128, fc, 32]),
                    op=TT.is_equal)
                dve_done(_i)
                dve_wait()
                _i = nc.vector.tensor_tensor(
                    out=mm[j].ap()[:, :fc, :], in0=oh[j].ap()[:, :fc, :],
                    in1=sg[j].ap()[:, :fc, 0::2], op=TT.mult)
                dve_done(_i)
                dve_wait()
                _i = nc.vector.reduce_sum(svr[j].ap()[:, :fc, 0:1],
                                          mm[j].ap()[:, :fc, :],
                                          axis=mybir.AxisListType.X)
                dve_done(_i)
                dve_wait()
                _i = nc.vector.tensor_tensor(
                    out=mm[j].ap()[:, :fc, :], in0=oh[j].ap()[:, :fc, :],
                    in1=sg[j].ap()[:, :fc, 1::2], op=TT.mult)
                dve_done(_i)
                dve_wait()
                _i = nc.vector.reduce_sum(svr[j].ap()[:, :fc, 1:2],
                                          mm[j].ap()[:, :fc, :],
                                          axis=mybir.AxisListType.X)
                dve_done(_i)
                # receiver select
                dve_wait()
                _i = nc.vector.tensor_tensor(
                    out=oh[j].ap()[:, :fc, :],
                    in0=rlot[j].ap()[:, :fc].unsqueeze(2).to_broadcast([128, fc, 32]),
                    in1=i32t.ap()[:, :].unsqueeze(1).to_broadcast([128, fc, 32]),
                    op=TT.is_equal)
                dve_done(_i)
                dve_wait()
                _i = nc.vector.tensor_tensor(
                    out=mm[j].ap()[:, :fc, :], in0=oh[j].ap()[:, :fc, :],
                    in1=rg[j].ap()[:, :fc, 0::2], op=TT.mult)
                dve_done(_i)
                dve_wait()
                _i = nc.vector.reduce_sum(rvr[j].ap()[:, :fc, 0:1],
                                          mm[j].ap()[:, :fc, :],
                                          axis=mybir.AxisListType.X)
                dve_done(_i)
                dve_wait()
                _i = nc.vector.tensor_tensor(
                    out=mm[j].ap()[:, :fc, :], in0=oh[j].ap()[:, :fc, :],
                    in1=rg[j].ap()[:, :fc, 1::2], op=TT.mult)
                dve_done(_i)
                dve_wait()
                _i = nc.vector.reduce_sum(
                    rvr[j].ap()[:, :fc, 1:2], mm[j].ap()[:, :fc, :],
                    axis=mybir.AxisListType.X)
                dve_done(_i)
                nc.gpsimd.wait_ge(vs, dvec[0])
                nc.gpsimd.dma_start(sv_all[:, c0:c0 + fc, :],
                                    svr[j].ap()[:, :fc, :]).then_inc(so, 16)
                nc.gpsimd.dma_start(rv_all[:, c0:c0 + fc, :],
                                    rvr[j].ap()[:, :fc, :]).then_inc(so, 16)
            nc.gpsimd.wait_ge(so, 32 * cfg.N_GT)
    nc.all_engine_barrier()

    # ---------------- edge phase (Tile) ----------------
    with tile.TileContext(nc) as tc, ExitStack() as ctx:
        consts = ctx.enter_context(tc.tile_pool(name="econsts", bufs=1))
        inp = ctx.enter_context(tc.tile_pool(name="einp", bufs=2))
        gat = ctx.enter_context(tc.tile_pool(name="egat", bufs=2))
        tmp = ctx.enter_context(tc.tile_pool(name="etmp", bufs=1))
        ohp = ctx.enter_context(tc.tile_pool(name="eoh", bufs=1))
        psum = ctx.enter_context(tc.tile_pool(name="epsum", bufs=1, space="PSUM"))
        psum2 = ctx.enter_context(tc.tile_pool(name="epsum2", bufs=1, space="PSUM"))

        ir = consts.tile([128, 128], I8)
        nc.sync.dma_start(ir[:], ir8_d[:])
        iq = consts.tile([128, cfg.QBINS], I8)
        nc.sync.dma_start(iq[:], iq8_d[:])
        idn = consts.tile([128, 128], F32)
        nc.sync.dma_start(idn[:], identf_d[:])
        eb = consts.tile([128, 4], F32)
        nc.sync.dma_start(eb[:], ebias[:])

        bins = psum.tile([128, cfg.QBINS], F32)

        n_mm = 0
        total_mm = cfg.C_TOT

        for t in range(n_tiles):
            c0 = t * F
            f = min(F, cfg.C_TOT - c0)
            lt16 = inp.tile([128, F], F16, name="lt16", tag="lt16")
            nc.sync.dma_start(lt16[:, :f], lens16[:, c0:c0 + f])
            mf = inp.tile([128, F], I8, name="mf", tag="mf")
            nc.sync.dma_start(mf[:, :f], m8d[:, c0:c0 + f])
            qf = inp.tile([128, F], I8, name="qf", tag="qf")
            nc.sync.dma_start(qf[:, :f], q8d[:, c0:c0 + f])

            sv = gat.tile([128, F, 2], F32, name="sv", tag="sv")
            nc.sync.dma_start(sv[:, :f, :], sv_all[:, c0:c0 + f, :])
            rv = gat.tile([128, F, 2], F32, name="rv", tag="rv")
            nc.sync.dma_start(rv[:, :f, :], rv_all[:, c0:c0 + f, :])

            als = sv[:, :f, 0]
            cs = sv[:, :f, 1]
            alr = rv[:, :f, 0]
            cr = rv[:, :f, 1]

            def T(tag):
                return tmp.tile([128, F], F32, name=tag, tag=tag)[:, :f]

            lt = T("ltf")
            nc.vector.tensor_copy(out=lt, in_=lt16[:, :f])

            a2 = T("a2"); nc.vector.tensor_add(out=a2, in0=als, in1=alr)
            u = T("u"); nc.vector.tensor_mul(out=u, in0=alr, in1=cs)
            tv = T("tv"); nc.vector.tensor_mul(out=tv, in0=als, in1=cr)
            ut = T("ut"); nc.vector.tensor_mul(out=ut, in0=u, in1=tv)
            du = T("du"); nc.vector.tensor_mul(out=du, in0=alr, in1=u)
            dt = T("dt"); nc.vector.tensor_mul(out=dt, in0=als, in1=tv)
            den = T("den"); nc.vector.tensor_add(out=den, in0=du, in1=dt)
            rden = T("rden"); nc.vector.reciprocal(out=rden, in_=den)
            c6p = T("c6p"); nc.vector.tensor_mul(out=c6p, in0=ut, in1=rden)

            la = T("la"); nc.scalar.activation(out=la, in_=a2, func=AF.Ln)
            q1 = T("q1"); nc.scalar.activation(out=q1, in_=la, func=AF.Exp,
                                               scale=1.0 / 7.0, bias=eb[:, 0:1])
            p6 = T("p6"); nc.scalar.activation(out=p6, in_=la, func=AF.Exp,
                                               scale=6.0 / 7.0, bias=eb[:, 1:2])
            p8 = T("p8"); nc.scalar.activation(out=p8, in_=la, func=AF.Exp,
                                               scale=8.0 / 7.0, bias=eb[:, 2:3])
            p10 = T("p10"); nc.scalar.activation(out=p10, in_=la, func=AF.Exp,
                                                 scale=10.0 / 7.0, bias=eb[:, 3:4])
            # s = b3 v^3 + b2 v^2 + b1 v + b0  (Horner)
            hh = T("hh"); nc.scalar.activation(out=hh, in_=q1, func=AF.Copy,
                                               scale=_GB3, bias=_GB2)
            h3 = T("h3"); nc.vector.tensor_mul(out=h3, in0=hh, in1=q1)
            nc.vector.tensor_scalar_add(out=h3, in0=h3, scalar1=_GB1)
            sres = T("sres"); nc.vector.tensor_mul(out=sres, in0=h3, in1=q1)
            nc.vector.tensor_scalar_add(out=sres, in0=sres, scalar1=_GB0)
            s2 = T("s2"); nc.vector.tensor_mul(out=s2, in0=sres, in1=sres)
            s4 = T("s4"); nc.vector.tensor_mul(out=s4, in0=s2, in1=s2)
            nc.vector.tensor_scalar_mul(out=s2, in0=s2, scalar1=10.0 * BOHR ** 2)
            nc.vector.tensor_scalar_mul(out=s4, in0=s4, scalar1=122.5 * BOHR ** 4)

            l2 = T("l2"); nc.vector.tensor_mul(out=l2, in0=lt, in1=lt)
            l4 = T("l4"); nc.vector.tensor_mul(out=l4, in0=l2, in1=l2)
            l6 = T("l6"); nc.vector.tensor_mul(out=l6, in0=l4, in1=l2)
            l8 = T("l8"); nc.vector.tensor_mul(out=l8, in0=l4, in1=l4)
            l10 = T("l10"); nc.vector.tensor_mul(out=l10, in0=l6, in1=l4)
            nc.vector.tensor_add(out=l6, in0=l6, in1=p6)
            nc.vector.tensor_add(out=l8, in0=l8, in1=p8)
            nc.vector.tensor_add(out=l10, in0=l10, in1=p10)
            r6 = T("r6"); nc.vector.reciprocal(out=r6, in_=l6)
            r8 = T("r8"); nc.vector.reciprocal(out=r8, in_=l8)
            r10 = T("r10"); nc.vector.reciprocal(out=r10, in_=l10)
            m8_ = T("m8_"); nc.vector.tensor_mul(out=m8_, in0=s2, in1=r8)
            m10 = T("m10"); nc.vector.tensor_mul(out=m10, in0=s4, in1=r10)
            nc.vector.tensor_add(out=r6, in0=r6, in1=m8_)
            nc.vector.tensor_add(out=r6, in0=r6, in1=m10)
            epre = T("epre"); nc.vector.tensor_mul(out=epre, in0=c6p, in1=r6)
            nc.vector.tensor_scalar_mul(out=epre, in0=epre,
                                        scalar1=-2.0 * _C6F)

            # switching function
            cx = T("cx"); nc.scalar.activation(out=cx, in_=lt, func=AF.Copy,
                                               scale=0.5, bias=-4.0)
            x1 = T("x1"); nc.scalar.activation(out=x1, in_=cx, func=AF.Copy,
                                               scale=-1.0, bias=1.0)
            nc.vector.tensor_scalar_max(out=x1, in0=x1, scalar1=1e-12)
            x2 = T("x2"); nc.vector.tensor_scalar_max(out=x2, in0=cx, scalar1=1e-12)
            n1 = T("n1"); nc.vector.reciprocal(out=n1, in_=x1)
            n2 = T("n2"); nc.vector.reciprocal(out=n2, in_=x2)
            nc.vector.tensor_scalar_min(out=n1, in0=n1, scalar1=87.0)
            nc.vector.tensor_scalar_min(out=n2, in0=n2, scalar1=87.0)
            e1 = T("e1"); nc.scalar.activation(out=e1, in_=n1, func=AF.Exp, scale=-1.0)
            e2 = T("e2"); nc.scalar.activation(out=e2, in_=n2, func=AF.Exp, scale=-1.0)
            ws = T("ws"); nc.vector.tensor_add(out=ws, in0=e1, in1=e2)
            nc.vector.tensor_scalar_add(out=ws, in0=ws, scalar1=1e-12)
            rw = T("rw"); nc.vector.reciprocal(out=rw, in_=ws)
            wv = T("wv"); nc.vector.tensor_mul(out=wv, in0=e1, in1=rw)
            v = T("v"); nc.vector.tensor_mul(out=v, in0=epre, in1=wv)

            # scatter: one-hot matmuls, batches of 32 columns
            BW = 32
            for b0 in range(0, f, BW):
                bw = min(BW, f - b0)
                ohr = ohp.tile([128, BW, 128], F32, name="ohr", tag="ohr")
                nc.vector.tensor_tensor(
                    out=ohr[:, :bw, :],
                    in0=mf[:, b0:b0 + bw].unsqueeze(2).to_broadcast([128, bw, 128]),
                    in1=ir[:].unsqueeze(1).to_broadcast([128, bw, 128]),
                    op=mybir.AluOpType.is_equal)
                ohq = ohp.tile([128, BW, cfg.QBINS], F32, name="ohq", tag="ohq")
                nc.vector.tensor_tensor(
                    out=ohq[:, :bw, :],
                    in0=qf[:, b0:b0 + bw].unsqueeze(2).to_broadcast(
                        [128, bw, cfg.QBINS]),
                    in1=iq[:].unsqueeze(1).to_broadcast([128, bw, cfg.QBINS]),
                    op=mybir.AluOpType.is_equal)
                nc.vector.tensor_tensor(
                    out=ohq[:, :bw, :],
                    in0=ohq[:, :bw, :],
                    in1=v[:, b0:b0 + bw].unsqueeze(2).to_broadcast(
                        [128, bw, cfg.QBINS]),
                    op=mybir.AluOpType.mult)
                for j in range(bw):
                    nc.tensor.matmul(
                        bins[:, :], lhsT=ohr[:, j, :], rhs=ohq[:, j, :],
                        start=(n_mm == 0), stop=(n_mm == total_mm - 1))
                    n_mm += 1

        # transpose bins [128, QBINS] -> [QBINS, 128] and write out
        bsb = consts.tile([128, cfg.QBINS], F32)
        nc.vector.tensor_copy(out=bsb[:], in_=bins[:])
        btp = psum2.tile([128, 128], F32)
        nc.tensor.transpose(out=btp[:cfg.QBINS, :], in_=bsb[:], identity=idn[:])
        bts = consts.tile([cfg.QBINS, 128], F32)
        nc.vector.tensor_copy(out=bts[:], in_=btp[:cfg.QBINS, :])
        nc.sync.dma_start(out[:, :], bts[:])

    nc.compile()
    return nc


_NC_CACHE = {}
_EXEC_CACHE = {}


def _get_nc(cfg):
    key = (cfg.N, cfg.C_TOT)
    if key not in _NC_CACHE:
        _NC_CACHE[key] = build_nc(cfg)
    return _NC_CACHE[key]


class _SpmdExec:
    """Cached shard_map execution of a Bass NEFF on n cores via PJRT.

    Mirrors concourse.bass2jax.run_bass_via_pjrt but keeps the jitted
    callable (and its lowering) across calls, so repeated runs only pay
    input transfer + device execution.
    """

    def __init__(self, nc, n_cores):
        from concourse import bass2jax
        bass2jax.install_neuronx_cc_hook()
        assert nc.dbg_addr is None or not nc.dbg_callbacks
        self.nc = nc
        self.n = n_cores
        partition_name = (nc.partition_id_tensor.name
                          if nc.partition_id_tensor else None)
        in_names, out_names, out_avals = [], [], []
        for alloc in nc.m.functions[0].allocations:
            if not isinstance(alloc, mybir.MemoryLocationSet):
                continue
            name = alloc.memorylocations[0].name
            if alloc.kind == "ExternalInput":
                if name != partition_name:
                    in_names.append(name)
            elif alloc.kind == "ExternalOutput":
                shape = tuple(alloc.tensor_shape)
                dtype = mybir.dt.np(alloc.dtype)
                out_names.append(name)
                out_avals.append(jax.core.ShapedArray(shape, dtype))
        self.dbg_name = nc.dbg_addr.name if nc.dbg_addr is not None else None
        self.in_names = list(in_names)
        if self.dbg_name is not None and self.dbg_name in self.in_names:
            pass  # already an ExternalInput; supplied as zeros in run()
        self.out_names = out_names
        self.out_avals = out_avals
        n_params = len(self.in_names)
        all_names = self.in_names + out_names
        if partition_name is not None:
            all_names.append(partition_name)
        donate = tuple(range(n_params, n_params + len(out_names)))
        bind_names = tuple(all_names)

        def _body(*args):
            operands = list(args)
            if partition_name is not None:
                operands.append(bass2jax.partition_id_tensor())
            outs = bass2jax._bass_exec_p.bind(
                *operands,
                out_avals=tuple(out_avals),
                in_names=bind_names,
                out_names=tuple(out_names),
                lowering_input_output_aliases=(),
                sim_require_finite=True,
                sim_require_nnan=True,
                nc=nc,
            )
            return tuple(outs)

        devices = jax.devices()[:n_cores]
        assert len(devices) == n_cores
        self.mesh = Mesh(np.asarray(devices), ("core",))
        in_specs = (PartitionSpec("core"),) * (n_params + len(out_names))
        out_specs = (PartitionSpec("core"),) * len(out_names)
        self.fn = jax.jit(
            shard_map(_body, mesh=self.mesh, in_specs=in_specs,
                      out_specs=out_specs, check_rep=False),
            donate_argnums=donate, keep_unused=True)

    def pack(self, in_maps):
        """Concatenate per-core maps to the global arrays the jit expects."""
        if self.dbg_name is not None:
            z = np.zeros((1, 2), np.uint32)
            in_maps = [{**m, self.dbg_name: z} for m in in_maps]
        return [
            np.concatenate([np.asarray(m[name]) for m in in_maps], axis=0)
            for name in self.in_names
        ]

    def run(self, packed):
        n = self.n
        zeros = [np.zeros((n * a.shape[0], *a.shape[1:]), a.dtype)
                 for a in self.out_avals]
        outs = self.fn(*packed, *zeros)
        return [
            {name: np.asarray(outs[i]).reshape(n, *self.out_avals[i].shape)[c]
             for i, name in enumerate(self.out_names)}
            for c in range(n)
        ]


def _get_exec(cfg):
    key = (cfg.N, cfg.C_TOT)
    if key not in _EXEC_CACHE:
        _EXEC_CACHE[key] = _SpmdExec(_get_nc(cfg), NCORES)
    return _EXEC_CACHE[key]


def shard_inputs(cfg, hirshfeld_ratios, atomic_numbers, senders_lr,
                 receivers_lr, lengths_lr):
    N, W, EPAD, C_TOT = cfg.N, cfg.W, cfg.EPAD, cfg.C_TOT
    h = np.asarray(hirshfeld_ratios, np.float32)
    z = np.asarray(atomic_numbers, np.int32)
    s = np.asarray(senders_lr, np.int32)
    r = np.asarray(receivers_lr, np.int32)
    ln = np.asarray(lengths_lr, np.float32)

    hp = np.ones(cfg.NPAD, np.float32)
    hp[:N] = h
    zp = np.ones(cfg.NPAD, np.int32)
    zp[:N] = z
    h16 = hp.reshape(128, cfg.NODE_F).astype(np.float16)
    z8 = (zp - 1).astype(np.int8).reshape(128, cfg.NODE_F).T.copy().reshape(-1)
    ac_tab = np.zeros((128, 2), np.float32)
    ac_tab[:len(ALPHAS), 0] = ALPHAS
    ac_tab[:len(C6_COEF), 1] = C6_COEF
    iota_col = np.arange(128, dtype=np.float32).reshape(128, 1)
    ebias = np.tile(np.array([[_B1, _B6, _B8, _B10]], np.float32), (128, 1))

    core_of = r // W
    order = np.argsort(core_of, kind="stable")
    s_o, r_o, l_o, c_o = s[order], r[order], ln[order], core_of[order]
    bounds = np.searchsorted(c_o, np.arange(NCORES + 1))

    def wrap_blk(arr):
        blk2 = (arr >> 5).astype(np.int16).reshape(128, C_TOT)
        parts = []
        for g in range(cfg.N_GT):
            c0 = 32 * g
            fc = min(32, C_TOT - c0)
            unw = blk2[:, c0:c0 + fc].T.reshape(-1)       # i = c*128+p
            parts.append(unw.reshape(fc * 8, 16).T)       # [16, fc*8]
        return np.ascontiguousarray(np.concatenate(parts, axis=1))

    in_maps = []
    for c in range(NCORES):
        lo, hi = bounds[c], bounds[c + 1]
        cnt = hi - lo
        assert cnt <= EPAD, f"core {c} edge count {cnt} > EPAD {EPAD}"
        base = c * W
        sp = np.zeros(EPAD, np.int32)
        rp = np.full(EPAD, base, np.int32)
        lp = np.full(EPAD, 100.0, np.float32)
        sp[:cnt] = s_o[lo:hi]
        rp[:cnt] = r_o[lo:hi]
        lp[:cnt] = l_o[lo:hi]
        rloc = rp - base
        in_maps.append({
            "h16": h16, "z8": z8, "ac_tab": ac_tab, "iota_col": iota_col,
            "ebias": ebias,
            "bs32": np.full((128, 1), base, np.int32),
            "sblk16": wrap_blk(sp), "rblk16": wrap_blk(rp),
            "slo8": (sp & 31).astype(np.int8).reshape(128, C_TOT),
            "m8": (rloc & 127).astype(np.int8).reshape(128, C_TOT),
            "q8": (rloc >> 7).astype(np.int8).reshape(128, C_TOT),
            "lens16": lp.astype(np.float16).reshape(128, C_TOT),
        })
    return in_maps


def unshard(cfg, results):
    outp = np.zeros(cfg.N, np.float32)
    for c in range(NCORES):
        o = results[c]["out"].reshape(-1)[:cfg.W]
        outp[c * cfg.W:(c + 1) * cfg.W] = o
    return outp.reshape(-1, 1)


def pack_inputs(cfg, in_maps):
    return _get_exec(cfg).pack(in_maps)


def run_all(cfg, packed):
    ex = _get_exec(cfg)
    return ex.run(packed)


def kernel(hirshfeld_ratios, atomic_numbers, senders_lr, receivers_lr,
           lengths_lr, num_nodes):
    cfg = FULL
    assert int(num_nodes) == cfg.N
    in_maps = shard_inputs(cfg, hirshfeld_ratios, atomic_numbers, senders_lr,
                           receivers_lr, lengths_lr)
    results = run_all(cfg, pack_inputs(cfg, in_maps))
    return unshard(cfg, results)
